# revision 1
# baseline (speedup 1.0000x reference)
"""Bass/Trainium2 kernel for BailingAttention (GQA prefill, causal, RoPE).

Sharding: tensor-parallel over heads across 8 NeuronCores. Each core computes
2 query heads + its group's shared KV head end-to-end (QKV projection, RoPE,
causal attention, output projection) and writes a partial [T, HID] output;
the host sums the 8 partials (the row-parallel all-reduce).

Layouts on device (partition dim first):
  hiddenT  [HID, T]   (host-transposed)  -> moving operand of QKV matmuls
  qT/kT    [D, T]     per head           -> RoPE applied in this layout
  v        [T, D]     natural            -> PV stationary (via PE transpose)
  scoresT  [kt, qt]   exp'd on ACT; denominator accumulated on PE via an
                      all-ones stationary (replicated column sums in PSUM)
  ctxT     [D, T]     -> stationary of the output projection

All matmuls run in fp32r (TF32-like: fp32 RNE-rounded to 11 mantissa bits)
at full PE speed. DRAM-sourced fp32r operands are pre-rounded bit-exactly on
the host so plain HWDGE DMAs suffice; on-device producers write fp32r
directly (the cast rounds).

The output projection for a 512-token block is emitted right after that
block's attention so its PSUM-evict copies and 1 MB output DMAs overlap the
next block's attention instead of running exposed at the end.
"""

import numpy as np

import concourse.bass as bass
import concourse.mybir as mybir
import concourse.tile as tile
from concourse import bacc, bass_utils
from concourse.bass import ts

F32 = mybir.dt.float32
F32R = mybir.dt.float32r
AF = mybir.ActivationFunctionType
OP = mybir.AluOpType

H, KV, D, HID, T = 16, 4, 128, 2048, 2048
THETA = 10000.0
N_CORES = 8
QH = H // N_CORES            # query heads per core = 2
TB = 512                     # token block (matmul moving N)
NTB = T // TB                # 4
HCN = HID // 128             # 16 h-chunks
NKT_TILES = T // 128         # 16 key tiles
SCALE = float(D) ** -0.5
PIPE = 3                     # attention software-pipeline depth (score MMs ahead)


def _to_f32r(a: np.ndarray) -> np.ndarray:
    """Round fp32 to fp32r bits (RNE to 11-bit mantissa) — bit-exactly what
    the hardware cast produces, so raw HWDGE DMA into f32r tiles is lossless."""
    b = np.ascontiguousarray(a, np.float32).view(np.uint32).astype(np.uint64)
    r = ((b + 0x7FF + ((b >> 12) & 1)) & 0xFFFFF000).astype(np.uint32)
    return r.view(np.float32)


def _build():
    nc = bacc.Bacc("TRN2", target_bir_lowering=False, debug=False,
                   num_devices=N_CORES)

    hT_d = nc.dram_tensor("hiddenT", [HID, T], F32R, kind="ExternalInput").ap()
    w_d = nc.dram_tensor("w_local", [HID, 4 * 128], F32R, kind="ExternalInput").ap()
    wo_d = nc.dram_tensor("wo_local", [2 * 128, HID], F32R, kind="ExternalInput").ap()
    cos_d = nc.dram_tensor("cosT", [128, T], F32, kind="ExternalInput").ap()
    sin_d = nc.dram_tensor("sinT", [128, T], F32, kind="ExternalInput").ap()
    mask_d = nc.dram_tensor("masks", [128, 4 * TB], F32, kind="ExternalInput").ap()
    ones_d = nc.dram_tensor("ones", [128, 128], F32R, kind="ExternalInput").ap()
    id_d = nc.dram_tensor("ident", [128, 128], F32, kind="ExternalInput").ap()
    out_d = nc.dram_tensor("out_partial", [T, HID], F32, kind="ExternalOutput").ap()

    with tile.TileContext(nc) as tc:
        with tc.tile_pool(name="const", bufs=1) as cpool, \
             tc.tile_pool(name="acts", bufs=1) as apool:
            # Resident constants. DMA emission order is load-bearing: the
            # QKV stream needs w-chunks + hT tiles first; everything else is
            # deferred so it doesn't delay the first matmuls.
            w_sb = cpool.tile([128, HCN, 512], F32R)
            wo_sb = cpool.tile([128, 2, HID], F32R)
            cos_sb = cpool.tile([128, T], F32)
            sin_sb = cpool.tile([128, T], F32)
            mask_sb = cpool.tile([128, 4, TB], F32)
            ones_sb = cpool.tile([128, 128], F32R)
            id_sb = cpool.tile([128, 128], F32)

            w_view = w_d.rearrange("(hc p) n -> hc p n", p=128)

            # persistent per-core activations
            qrT = [apool.tile([128, T], F32R, name=f"qrT{i}", tag=f"qrT{i}")
                   for i in range(QH)]
            krT = apool.tile([128, T], F32R)
            v_nat = apool.tile([128, NKT_TILES, 128], F32R)
            ctxT = [apool.tile([128, T], F32R, name=f"ctxT{i}", tag=f"ctxT{i}")
                    for i in range(QH)]

            hT_view = hT_d.rearrange("(hc p) t -> hc p t", p=128)

            # ================= Phase 1: QKV projection (+RoPE, v transpose) ==
            with tc.tile_pool(name="hstream", bufs=8) as hpool, \
                 tc.tile_pool(name="p1tmp", bufs=3) as tpool, \
                 tc.tile_pool(name="p1psum", bufs=1, space="PSUM") as qkv_ps_pool, \
                 tc.tile_pool(name="p1psumv", bufs=2, space="PSUM") as vps_pool:
                for b in range(NTB):
                    ps_qkv = [qkv_ps_pool.tile([128, TB], F32, name=f"psqkv{n}",
                                               tag=f"qkv{n}") for n in range(4)]
                    for hc in range(HCN):
                        if b == 0:
                            nc.sync.dma_start(w_sb[:, hc, :], w_view[hc])
                        hT_t = hpool.tile([128, TB], F32R)
                        nc.sync.dma_start(hT_t[:], hT_view[hc, :, ts(b, TB)])
                        for n in range(4):
                            nc.tensor.matmul(ps_qkv[n][:], w_sb[:, hc, ts(n, 128)],
                                             hT_t[:], start=(hc == 0),
                                             stop=(hc == HCN - 1))
                    if b == 0:
                        nc.scalar.dma_start(id_sb[:], id_d)
                        nc.scalar.dma_start(ones_sb[:], ones_d)
                    nc.scalar.dma_start(cos_sb[:, ts(b, TB)], cos_d[:, ts(b, TB)])
                    nc.scalar.dma_start(sin_sb[:, ts(b, TB)], sin_d[:, ts(b, TB)])
                    # Evict all four accumulators first (frees PSUM for the
                    # next block's matmuls), then RoPE / v-transpose.
                    x_sbs = []
                    for n in range(4):
                        x_sb = tpool.tile([128, TB], F32, tag=f"ropex{n}",
                                          name=f"x_sb{n}")
                        if n % 2 == 0:
                            nc.scalar.copy(x_sb[:], ps_qkv[n][:])
                        else:
                            nc.vector.tensor_copy(x_sb[:], ps_qkv[n][:])
                        x_sbs.append(x_sb)
                    for n in range(3):
                        dst = qrT[n] if n < QH else krT
                        x_sb = x_sbs[n]
                        xsw = tpool.tile([128, TB], F32, tag="ropesw")
                        nc.scalar.dma_start(xsw[0:64, :], x_sb[64:128, :])
                        nc.scalar.dma_start(xsw[64:128, :], x_sb[0:64, :])
                        t2 = tpool.tile([128, TB], F32, tag="ropet2")
                        nc.gpsimd.tensor_tensor(out=t2[:], in0=xsw[:],
                                                in1=sin_sb[:, ts(b, TB)], op=OP.mult)
                        m1 = tpool.tile([128, TB], F32, tag="ropem1")
                        nc.gpsimd.tensor_tensor(out=m1[:], in0=x_sb[:],
                                                in1=cos_sb[:, ts(b, TB)], op=OP.mult)
                        nc.vector.tensor_tensor(out=dst[:, ts(b, TB)], in0=m1[:],
                                                in1=t2[:], op=OP.add)
                    vT_sb = x_sbs[3]
                    for j in range(4):
                        ps_v = vps_pool.tile([128, 128], F32)
                        nc.tensor.transpose(ps_v[:], vT_sb[:, ts(j, 128)], id_sb[:])
                        nc.vector.tensor_copy(v_nat[:, 4 * b + j, :], ps_v[:])
                    if b == 2:
                        nc.scalar.dma_start(
                            mask_sb[:], mask_d.rearrange("p (m n) -> p m n", n=TB))
                        nc.scalar.dma_start(
                            wo_sb[:], wo_d.rearrange("(c p) n -> p c n", p=128))

            # ============ Phase 2+3: causal attention + output projection ====
            # Per 512-token block: attention for both heads, then that block's
            # output projection (its copies/DMAs overlap the next block).
            with tc.tile_pool(name="p2exp", bufs=PIPE + 5) as epool, \
                 tc.tile_pool(name="p2tmp", bufs=2) as t2pool, \
                 tc.tile_pool(name="p3out", bufs=2) as opool, \
                 tc.tile_pool(name="p2ps_s", bufs=PIPE + 1, space="PSUM") as sps_pool, \
                 tc.tile_pool(name="p2ps_c", bufs=1, space="PSUM") as cps_pool, \
                 tc.tile_pool(name="p2ps_d", bufs=1, space="PSUM") as dps_pool, \
                 tc.tile_pool(name="p3psum", bufs=2, space="PSUM") as ops_pool:
                for b in range(NTB):
                    nkt = 4 * (b + 1)
                    for qh in range(QH):
                        ctx_ps = cps_pool.tile([128, TB], F32, name="ctx_ps")
                        den_ps = dps_pool.tile([128, TB], F32, name="den_ps")
                        e_tiles = [None] * nkt

                        def emit_score(kt, b=b, qh=qh, e_tiles=e_tiles):
                            s_ps = sps_pool.tile([128, TB], F32, name="s_ps")
                            nc.tensor.matmul(s_ps[:], krT[:, ts(kt, 128)],
                                             qrT[qh][:, ts(b, TB)],
                                             start=True, stop=True)
                            e_sb = epool.tile([128, TB], F32R, name="e_sb",
                                              tag="exp")
                            nc.scalar.activation(e_sb[:], s_ps[:], AF.Exp,
                                                 scale=SCALE)
                            if kt >= 4 * b:   # diagonal tile: causal mask
                                nc.vector.tensor_tensor(
                                    out=e_sb[:], in0=e_sb[:],
                                    in1=mask_sb[:, kt - 4 * b, :], op=OP.mult)
                            e_tiles[kt] = e_sb

                        def emit_consume(kt, nkt=nkt, ctx_ps=ctx_ps,
                                         den_ps=den_ps, e_tiles=e_tiles):
                            e_sb = e_tiles[kt]
                            nc.tensor.matmul(ctx_ps[:], v_nat[:, kt, :], e_sb[:],
                                             start=(kt == 0), stop=(kt == nkt - 1))
                            nc.tensor.matmul(den_ps[:], ones_sb[:], e_sb[:],
                                             start=(kt == 0), stop=(kt == nkt - 1))

                        # Head 1's consumes start deeper so its score MMs
                        # cover head 0's recip/normalize chain (the single
                        # ctx PSUM bank frees only after that chain).
                        depth = PIPE if qh == 0 else min(PIPE + 2, nkt)
                        for kt in range(nkt + depth):
                            if kt < nkt:
                                emit_score(kt)
                            if kt >= depth:
                                emit_consume(kt - depth)

                        recip = t2pool.tile([128, TB], F32, tag="recip",
                                            name="recip")
                        nc.vector.reciprocal(recip[:], den_ps[:])
                        nc.vector.tensor_tensor(out=ctxT[qh][:, ts(b, TB)],
                                                in0=ctx_ps[:], in1=recip[:],
                                                op=OP.mult)

                    # ---- output projection for this block's 4 token tiles ----
                    for tt in range(4 * b, 4 * b + 4):
                        o_sb = opool.tile([128, HID], F32, name="o_sb")
                        for n in range(4):
                            ps_o = ops_pool.tile([128, 512], F32, name="ps_o")
                            for qh in range(QH):
                                nc.tensor.matmul(ps_o[:], ctxT[qh][:, ts(tt, 128)],
                                                 wo_sb[:, qh, ts(n, 512)],
                                                 start=(qh == 0),
                                                 stop=(qh == QH - 1))
                            if n == 0:
                                nc.scalar.copy(o_sb[:, ts(n, 512)], ps_o[:])
                            else:
                                nc.vector.tensor_copy(o_sb[:, ts(n, 512)], ps_o[:])
                        nc.sync.dma_start(out_d[ts(tt, 128), :], o_sb[:])

    nc.compile()
    return nc


_NC_CACHE = None


def _get_nc():
    global _NC_CACHE
    if _NC_CACHE is None:
        _NC_CACHE = _build()
    return _NC_CACHE


def _host_tables(position_ids: np.ndarray):
    pos = np.asarray(position_ids, np.float32)
    inv_freq = (1.0 / (THETA ** (np.arange(0, D, 2, dtype=np.float32) / D)))
    ang = pos[:, None] * inv_freq[None, :]          # [T, 64] f32
    cos = np.cos(ang).T.astype(np.float32)          # [64, T]
    sin = np.sin(ang).T.astype(np.float32)
    cosT = np.concatenate([cos, cos], axis=0)       # [128, T]
    sinT = np.concatenate([-sin, sin], axis=0)
    return cosT, sinT


def _host_masks():
    r = np.arange(128)[:, None]
    c = np.arange(TB)[None, :]
    m = [(c - r - 128 * i >= 0).astype(np.float32) for i in range(4)]
    return np.concatenate(m, axis=1)                # [128, 4*TB]


def kernel(hidden_states, position_ids, Wqkv, Wo):
    hidden_states = np.asarray(hidden_states, np.float32)
    Wqkv = np.asarray(Wqkv, np.float32)
    Wo = np.asarray(Wo, np.float32)

    nc = _get_nc()

    hiddenT = _to_f32r(hidden_states.T)
    cosT, sinT = _host_tables(position_ids)
    masks = _host_masks()
    ones = np.ones((128, 128), np.float32)
    ident = np.eye(128, dtype=np.float32)

    wq = Wqkv[:, : H * D]
    wk = Wqkv[:, H * D: (H + KV) * D]
    wv = Wqkv[:, (H + KV) * D:]

    in_maps = []
    for c in range(N_CORES):
        kvh = (c * QH) // (H // KV)
        w_local = np.concatenate(
            [wq[:, (c * QH) * D: (c * QH + 1) * D],
             wq[:, (c * QH + 1) * D: (c * QH + 2) * D],
             wk[:, kvh * D: (kvh + 1) * D],
             wv[:, kvh * D: (kvh + 1) * D]], axis=1)
        wo_local = Wo[c * QH * D: (c + 1) * QH * D, :]
        in_maps.append({
            "hiddenT": hiddenT,
            "w_local": _to_f32r(w_local),
            "wo_local": _to_f32r(wo_local),
            "cosT": cosT, "sinT": sinT, "masks": masks,
            "ones": ones, "ident": ident,
        })

    res = bass_utils.run_bass_kernel_spmd(nc, in_maps,
                                          core_ids=list(range(N_CORES)))
    parts = np.stack([res.results[c]["out_partial"] for c in range(N_CORES)], 0)
    return parts.sum(axis=0, dtype=np.float32)



# revision 29
# speedup vs baseline: 1.2918x; 1.2918x over previous
"""Bass/Trainium2 kernel for BailingAttention (GQA prefill, causal, RoPE).

Sharding: tensor-parallel over heads across 8 NeuronCores. Each core computes
2 query heads + its group's shared KV head end-to-end (QKV projection, RoPE,
causal attention, output projection) and writes a partial [T, HID] fp16
output; the host sums the 8 partials (the row-parallel all-reduce).

Precision plan (gate is rel-err < 2e-2; this lands ~1.4e-2):
  - QKV + output projections: fp16 x fp16 matmuls (1 PE cycle/row).
  - Scores: fp8 DoubleRow, one instruction per key tile computing
    k8^T(q_hi + q_lo) -- the two DoubleRow subtile slots carry a hi/lo fp8
    split of q, so the q side is ~exact and only k carries fp8 noise.
    0.5 cycles/row: 2x fp32r.
  - exp: ACT engine writes fp8e4 directly; softmax numerator/denominator use
    the SAME quantized e so weight-quantization largely cancels.
  - PV: two DoubleRow instructions per key-tile PAIR: (v_hi[2j],v_hi[2j+1])
    and (v_lo[...]) against the e pair -- v is hi/lo-split (~exact), e noise
    cancels through the denominator. 2x fp32r.
  - Denominator: DoubleRow over e pairs with a ones stationary: 4x fp32r.

Schedule: one fused software pipeline. Block 0's QKV runs first; thereafter
attention for block i runs with the QKV matmuls of block i+1 and the
output-projection units of block i-1 woven between its score/PV steps as
tensor-engine filler, so the PE stays dense (and at full p-state clock)
while ACT works through the exps. All of block i+1's hidden-stream DMAs are
prefetched at the start of attention i; RoPE for block i+1 is emitted as
soon as its last hc lands so its latency hides under block i's remaining
pairs. Output-projection units rotate over four PSUM banks (the scratch
bank plus the three QKV banks, which are idle between accumulations).

Layouts on device (partition dim first):
  hT16    [HID, T] fp16 (host-transposed)  -> moving operand of QKV matmuls
  q8      [D, 2(hi,lo), T] fp8 per head    -> scores moving
  k8      [D, 2(dup), T] fp8               -> scores stationary slices
  v hi/lo [T-part, kt, D] fp8 natural      -> PV stationary; produced by
          per-token-tile matmuls (stationary = hT slice) -- no PE transpose
  e8      [kt, 2, TB] fp8 pair tiles       -> PV/denominator moving
  ctxT    [D, T] fp16                      -> output projection stationary
"""

import numpy as np
import ml_dtypes

import concourse.bass as bass
import concourse.mybir as mybir
import concourse.tile as tile
from concourse import bacc, bass_utils
from concourse.bass import ts

F32 = mybir.dt.float32
F16 = mybir.dt.float16
F8 = mybir.dt.float8e4
AF = mybir.ActivationFunctionType
OP = mybir.AluOpType
DR = mybir.MatmulPerfMode.DoubleRow

H, KV, D, HID, T = 16, 4, 128, 2048, 2048
THETA = 10000.0
N_CORES = 8
QH = H // N_CORES            # query heads per core = 2
TB = 512                     # token block
NTB = T // TB                # 4
HCN = HID // 128             # 16 h-chunks
SCALE = float(D) ** -0.5
F8NP = ml_dtypes.float8_e4m3

# hc group layout: block 0 ramps up (small first DMA so the first matmul
# starts early); other blocks use 4-chunk groups
GROUPS0 = [(0, 1), (1, 3), (4, 4), (8, 4), (12, 4)]
GROUPS = [(0, 4), (4, 4), (8, 4), (12, 4)]


def _build():
    nc = bacc.Bacc("TRN2", target_bir_lowering=False, debug=False,
                   num_devices=N_CORES)

    hT_d = nc.dram_tensor("hT16", [HID, T], F16, kind="ExternalInput").ap()
    w_d = nc.dram_tensor("w16", [128, HCN, 4 * 128], F16, kind="ExternalInput").ap()
    cs_d = nc.dram_tensor("cs16", [128, 2, T], F16, kind="ExternalInput").ap()
    mask_d = nc.dram_tensor("mask16", [128, 4, TB], F16, kind="ExternalInput").ap()
    ones_d = nc.dram_tensor("ones8", [128, 2, 128], F8, kind="ExternalInput").ap()
    wo_d = nc.dram_tensor("wo16", [128, 2, HID], F16, kind="ExternalInput").ap()
    out_d = nc.dram_tensor("out16", [T, HID], F16, kind="ExternalOutput").ap()

    hT_view = hT_d.rearrange("(hc p) t -> hc p t", p=128)

    with tile.TileContext(nc) as tc:
        with tc.tile_pool(name="const", bufs=1) as cpool, \
             tc.tile_pool(name="acts", bufs=1) as apool, \
             tc.tile_pool(name="hstream", bufs=8) as hpool, \
             tc.tile_pool(name="rtmp", bufs=3) as tpool, \
             tc.tile_pool(name="p2e", bufs=6) as epool, \
             tc.tile_pool(name="p2tmp", bufs=2) as t2pool, \
             tc.tile_pool(name="p3out", bufs=2) as opool, \
             tc.tile_pool(name="qkvps", bufs=1, space="PSUM") as qkvps, \
             tc.tile_pool(name="scrps", bufs=1, space="PSUM") as scrps, \
             tc.tile_pool(name="sps", bufs=1, space="PSUM") as sps_pool, \
             tc.tile_pool(name="ops", bufs=1, space="PSUM") as ops_pool, \
             tc.tile_pool(name="cps", bufs=1, space="PSUM") as cps_pool, \
             tc.tile_pool(name="dps", bufs=1, space="PSUM") as dps_pool:

            w_sb = cpool.tile([128, HCN, 4 * 128], F16)
            cs_sb = cpool.tile([128, 2, T], F16)
            mask_sb = cpool.tile([128, 4, TB], F16)
            ones_sb = cpool.tile([128, 2, 128], F8)
            wo_sb = cpool.tile([128, 2, HID], F16)

            q8 = [apool.tile([128, 2, T], F8, name=f"q8_{i}", tag=f"q8_{i}")
                  for i in range(QH)]
            k8 = apool.tile([128, 2, T], F8)
            vhi = apool.tile([128, 16, 128], F8)
            vlo = apool.tile([128, 16, 128], F8)
            ctxT = [apool.tile([128, T], F16, name=f"ctxT{i}", tag=f"ctxT{i}")
                    for i in range(QH)]

            # ---------------- emit helpers ----------------
            qkv_live = {}   # block -> [ps_q0, ps_q1, ps_k]
            vnat_live = {}  # block -> ps_v (from scratch pool)
            h_tiles = {}    # (block, group-idx) -> sbuf tile
            h_keep = {}     # retained h tiles for deferred v-nat

            def emit_h_dma(i, gi):
                hc0, wid = (GROUPS0 if i == 0 else GROUPS)[gi]
                hT_t = hpool.tile([128, wid, TB], F16, tag=f"h{wid}")
                src = hT_view[hc0:hc0 + wid, :, ts(i, TB)]
                nc.sync.dma_start(hT_t[:], src.rearrange("g p t -> p g t"))
                h_tiles[(i, gi)] = hT_t

            def emit_qkv_group(i, gi):
                hc0, wid = (GROUPS0 if i == 0 else GROUPS)[gi]
                if gi == 0:
                    qkv_live[i] = [qkvps.tile([128, TB], F32, name=f"psqkv{n}",
                                              tag=f"qkv{n}") for n in range(3)]
                ps_qkv = qkv_live[i]
                hT_t = h_tiles.pop((i, gi))
                h_keep[(i, gi)] = hT_t
                for j in range(wid):
                    hc = hc0 + j
                    for n in range(3):
                        nc.tensor.matmul(ps_qkv[n][:], w_sb[:, hc, ts(n, 128)],
                                         hT_t[:, j, :], start=(hc == 0),
                                         stop=(hc == HCN - 1))

            def emit_vnat(i):
                # v in natural layout from the retained h tiles. tt-OUTER:
                # each PSUM sub-region's accumulation group must run
                # start->stop consecutively (interleaved same-bank
                # accumulation groups produce garbage on hardware).
                vnat_live[i] = scrps.tile([128, 4, 128], F32, name="scr",
                                          tag="scr")
                ps_v = vnat_live[i]
                groups = GROUPS0 if i == 0 else GROUPS
                for tt in range(4):
                    for gi, (hc0, wid) in enumerate(groups):
                        hT_t = h_keep[(i, gi)]
                        for j in range(wid):
                            hc = hc0 + j
                            nc.tensor.matmul(ps_v[:, tt, :],
                                             hT_t[:, j, ts(tt, 128)],
                                             w_sb[:, hc, ts(3, 128)],
                                             start=(hc == 0),
                                             stop=(hc == HCN - 1))
                for gi in range(len(groups)):
                    h_keep.pop((i, gi))
                # v hi/lo casts
                nc.vector.tensor_copy(vhi[:, ts(i, 4), :], ps_v[:])
                nc.vector.tensor_tensor(out=vlo[:, ts(i, 4), :], in0=ps_v[:],
                                        in1=vhi[:, ts(i, 4), :], op=OP.subtract)
                vnat_live.pop(i)

            def emit_rope(i):
                ps_qkv = qkv_live.pop(i)
                # fp16 RoPE; k chunk (slot 2) first so scores can start
                # after k + q0 land
                x_all = tpool.tile([128, 3, TB], F16, tag="ropex")
                for n in (2, 0, 1):
                    nc.vector.tensor_copy(x_all[:, n, :], ps_qkv[n][:])
                xsw = tpool.tile([128, 3, TB], F16, tag="ropesw")
                nc.scalar.dma_start(xsw[0:64, :, :], x_all[64:128, :, :])
                nc.scalar.dma_start(xsw[64:128, :, :], x_all[0:64, :, :])
                for n in (2, 0, 1):
                    t2 = tpool.tile([128, TB], F16, tag="ropet2")
                    nc.vector.tensor_tensor(out=t2[:], in0=xsw[:, n, :],
                                            in1=cs_sb[:, 1, ts(i, TB)],
                                            op=OP.mult)
                    m1 = tpool.tile([128, TB], F16, tag="ropem1")
                    nc.gpsimd.tensor_tensor(out=m1[:], in0=x_all[:, n, :],
                                            in1=cs_sb[:, 0, ts(i, TB)],
                                            op=OP.mult)
                    if n < QH:
                        qf = tpool.tile([128, TB], F16, tag="ropeqf")
                        nc.vector.tensor_tensor(out=qf[:], in0=m1[:], in1=t2[:],
                                                op=OP.add)
                        nc.gpsimd.tensor_copy(q8[n][:, 0, ts(i, TB)], qf[:])
                        nc.vector.tensor_tensor(
                            out=q8[n][:, 1, ts(i, TB)], in0=qf[:],
                            in1=q8[n][:, 0, ts(i, TB)], op=OP.subtract)
                    else:
                        nc.vector.tensor_tensor(out=k8[:, 0, ts(i, TB)],
                                                in0=m1[:], in1=t2[:], op=OP.add)
                        nc.scalar.copy(k8[:, 1, ts(i, TB)], k8[:, 0, ts(i, TB)])

            pending = []    # outproj units (tt, n) awaiting emission
            osb_map = {}
            unit_ctr = [0]
            TAIL_TAGS = ["ops", "qkv0", "qkv1", "qkv2"]

            def emit_outproj_unit(tail=False, dve_only=False):
                tt, n = pending.pop(0)
                if tt not in osb_map:
                    osb_map[tt] = opool.tile([128, HID], F16, name="o_sb")
                o_sb = osb_map[tt]
                unit_ctr[0] += 1
                tag = TAIL_TAGS[unit_ctr[0] % 4] if tail else "ops"
                if tag == "ops":
                    ps_o = ops_pool.tile([128, TB], F32, name="ps_o", tag="ops")
                else:
                    ps_o = qkvps.tile([128, TB], F32, name=f"ps{tag}", tag=tag)
                nc.tensor.matmul(ps_o[:], ctxT[0][:, ts(tt, 128)],
                                 wo_sb[:, 0, ts(n, 512)], start=True, stop=False)
                nc.tensor.matmul(ps_o[:], ctxT[1][:, ts(tt, 128)],
                                 wo_sb[:, 1, ts(n, 512)], start=False, stop=True)
                if dve_only or unit_ctr[0] % 2 == 0:
                    nc.vector.tensor_copy(o_sb[:, ts(n, 512)], ps_o[:])
                else:
                    nc.scalar.copy(o_sb[:, ts(n, 512)], ps_o[:])
                if n == 3:
                    nc.sync.dma_start(out_d[ts(tt, 128), :], o_sb[:])
                    del osb_map[tt]

            def emit_attn_block(i):
                """Attention for block i, weaving in QKV matmuls of block
                i+1 and outproj units of block i-1 as PE filler."""
                ngrp = len(GROUPS) if i < NTB - 1 else 0
                grp_q = list(range(ngrp))
                if i < NTB - 1:
                    for gi in range(ngrp):
                        emit_h_dma(i + 1, gi)
                    nc.sync.dma_start(cs_sb[:, :, ts(i + 1, TB)],
                                      cs_d[:, :, ts(i + 1, TB)])
                if i == 0:
                    nc.sync.dma_start(mask_sb[:], mask_d)
                    nc.sync.dma_start(ones_sb[:], ones_d)
                    nc.sync.dma_start(wo_sb[:], wo_d)
                if i == 0:
                    # block 0's v-nat runs here, out of its DMA-bound QKV
                    # stream, covering RoPE(0)'s latency tail
                    emit_vnat(0)
                npair = 2 * (i + 1)
                steps_total = npair * QH
                step = [0]

                def filler():
                    steps_left = steps_total - step[0]
                    step[0] += 1
                    if grp_q:
                        emit_qkv_group(i + 1, grp_q.pop(0))
                        if not grp_q:
                            emit_vnat(i + 1)
                            emit_rope(i + 1)
                        return
                    if pending and steps_left > 0:
                        n_fill = (len(pending) + steps_left - 1) // steps_left
                        for _ in range(min(n_fill, len(pending))):
                            emit_outproj_unit(tail=(i == NTB - 1),
                                              dve_only=(i == NTB - 1))

                for qh in range(QH):
                    ctx_ps = cps_pool.tile([128, TB], F32, name="ctx_ps")
                    den_ps = dps_pool.tile([128, TB], F32, name="den_ps")
                    nkt = 4 * (i + 1)
                    e_tiles = [None] * npair

                    def emit_score_kt(kt, i=i, qh=qh, e_tiles=e_tiles):
                        pj, si = kt // 2, kt % 2
                        if si == 0:
                            e_tiles[pj] = epool.tile([128, 2, TB], F8,
                                                     name="e8", tag="e8")
                        e8t = e_tiles[pj]
                        if i == NTB - 1:
                            stag = ["s_ps", "qkv0", "qkv1", "qkv2"][kt % 4]
                        else:
                            stag = "s_ps"
                        if stag == "s_ps":
                            s_ps = sps_pool.tile([128, TB], F32, name="s_ps")
                        else:
                            s_ps = qkvps.tile([128, TB], F32, name=stag,
                                              tag=stag)
                        j = kt - 4 * i
                        lo = 128 * j if j > 0 else 0
                        nc.tensor.matmul(s_ps[:, lo:TB], k8[:, :, ts(kt, 128)],
                                         q8[qh][:, :, i * TB + lo:
                                                (i + 1) * TB],
                                         start=True, stop=True, perf_mode=DR)
                        nc.scalar.activation(e8t[:, si, lo:TB],
                                             s_ps[:, lo:TB], AF.Exp,
                                             scale=SCALE)
                        if j >= 0:
                            if lo > 0:
                                # the skipped prefix holds stale pool bytes
                                # (can be fp8 NaN/Inf -- x*0 would keep NaN):
                                # zero it explicitly on the idle Pool engine
                                nc.gpsimd.memset(e8t[:, si, 0:lo], 0)
                            nc.vector.tensor_tensor(
                                out=e8t[:, si, lo:lo + 128],
                                in0=e8t[:, si, lo:lo + 128],
                                in1=mask_sb[:, j, lo:lo + 128], op=OP.mult)

                    def emit_consume_piece(ck, npair=npair, ctx_ps=ctx_ps,
                                           den_ps=den_ps, e_tiles=e_tiles):
                        pj = ck // 2
                        e8t = e_tiles[pj]
                        if ck % 2 == 0:
                            nc.tensor.matmul(ctx_ps[:], vhi[:, ts(pj, 2), :],
                                             e8t[:], start=(pj == 0),
                                             stop=False, perf_mode=DR)
                        else:
                            nc.tensor.matmul(ctx_ps[:], vlo[:, ts(pj, 2), :],
                                             e8t[:], start=False,
                                             stop=(pj == npair - 1),
                                             perf_mode=DR)
                            nc.tensor.matmul(den_ps[:], ones_sb[:], e8t[:],
                                             start=(pj == 0),
                                             stop=(pj == npair - 1),
                                             perf_mode=DR)

                    LAG = 3
                    for k in range(nkt + LAG):
                        if k < nkt:
                            emit_score_kt(k)
                        ck = k - LAG
                        if ck >= 0:
                            emit_consume_piece(ck)
                            if ck % 2 == 1:
                                filler()

                    recip = t2pool.tile([128, TB], F32, tag="recip",
                                        name="recip")
                    nc.vector.reciprocal(recip[:], den_ps[:])
                    nc.vector.tensor_tensor(out=ctxT[qh][:, ts(i, TB)],
                                            in0=ctx_ps[:], in1=recip[:],
                                            op=OP.mult)

            # ---------------- main pipeline ----------------
            emit_h_dma(0, 0)
            nc.sync.dma_start(w_sb[:, 0:1, :], w_d[:, 0:1, :])
            emit_h_dma(0, 1)
            nc.sync.dma_start(w_sb[:, 1:4, :], w_d[:, 1:4, :])
            nc.sync.dma_start(cs_sb[:, :, ts(0, TB)], cs_d[:, :, ts(0, TB)])
            emit_h_dma(0, 2)
            nc.sync.dma_start(w_sb[:, 4:8, :], w_d[:, 4:8, :])
            emit_h_dma(0, 3)
            nc.sync.dma_start(w_sb[:, 8:12, :], w_d[:, 8:12, :])
            emit_h_dma(0, 4)
            nc.sync.dma_start(w_sb[:, 12:16, :], w_d[:, 12:16, :])
            for gi in range(len(GROUPS0)):
                emit_qkv_group(0, gi)
            emit_rope(0)
            for i in range(NTB):
                emit_attn_block(i)
                pending.extend((tt, n) for tt in range(4 * i, 4 * i + 4)
                               for n in range(4))
            while pending:
                emit_outproj_unit(tail=True)

    nc.compile()
    return nc


_NC_CACHE = None


def _get_nc():
    global _NC_CACHE
    if _NC_CACHE is None:
        _NC_CACHE = _build()
    return _NC_CACHE


def _host_tables(position_ids: np.ndarray):
    pos = np.asarray(position_ids, np.float32)
    inv_freq = (1.0 / (THETA ** (np.arange(0, D, 2, dtype=np.float32) / D)))
    ang = pos[:, None] * inv_freq[None, :]          # [T, 64] f32
    cos = np.cos(ang).T                             # [64, T]
    sin = np.sin(ang).T
    cosT = np.concatenate([cos, cos], axis=0).astype(np.float16)
    sinT = np.concatenate([-sin, sin], axis=0).astype(np.float16)
    return cosT, sinT


def _host_masks():
    # mask for diagonal tile j (keys 128j..128j+128 of the block): columns
    # [0, 128(j+1)): zero where q < k, i.e. col < 128j + row
    r = np.arange(128)[:, None]
    c = np.arange(TB)[None, :]
    m = np.stack([(c - r - 128 * j >= 0) for j in range(4)], axis=1)
    return m.astype(np.float16)                     # [128, 4, TB]


def kernel(hidden_states, position_ids, Wqkv, Wo):
    hidden_states = np.asarray(hidden_states, np.float32)
    Wqkv = np.asarray(Wqkv, np.float32)
    Wo = np.asarray(Wo, np.float32)

    nc = _get_nc()

    hT16 = np.ascontiguousarray(hidden_states.T).astype(np.float16)
    cosT, sinT = _host_tables(position_ids)
    cs16 = np.ascontiguousarray(np.stack([cosT, sinT], axis=1))  # [128,2,T]
    masks = _host_masks()
    ones8 = np.ones((128, 2, 128), dtype=F8NP)

    wq = Wqkv[:, : H * D]
    wk = Wqkv[:, H * D: (H + KV) * D]
    wv = Wqkv[:, (H + KV) * D:]

    in_maps = []
    for c in range(N_CORES):
        kvh = (c * QH) // (H // KV)
        w_cols = np.concatenate(
            [wq[:, (c * QH) * D: (c * QH + 1) * D],
             wq[:, (c * QH + 1) * D: (c * QH + 2) * D],
             wk[:, kvh * D: (kvh + 1) * D],
             wv[:, kvh * D: (kvh + 1) * D]], axis=1)         # [HID, 512]
        w16 = np.ascontiguousarray(
            w_cols.reshape(HCN, 128, 4 * 128).transpose(1, 0, 2)
        ).astype(np.float16)                                 # [128, HCN, 512]
        wo_local = Wo[c * QH * D: (c + 1) * QH * D, :]       # [256, HID]
        wo16 = np.ascontiguousarray(
            wo_local.reshape(2, 128, HID).transpose(1, 0, 2)
        ).astype(np.float16)                                 # [128, 2, HID]
        in_maps.append({
            "hT16": hT16, "w16": w16, "cs16": cs16,
            "mask16": masks, "ones8": ones8, "wo16": wo16,
        })

    res = bass_utils.run_bass_kernel_spmd(nc, in_maps,
                                          core_ids=list(range(N_CORES)))
    parts = np.stack([res.results[c]["out16"].astype(np.float32)
                      for c in range(N_CORES)], 0)
    return parts.sum(axis=0, dtype=np.float32)


# revision 41
# speedup vs baseline: 1.3058x; 1.0108x over previous
"""Bass/Trainium2 kernel for BailingAttention (GQA prefill, causal, RoPE).

Sharding: tensor-parallel over heads across 8 NeuronCores. Each core computes
2 query heads + its group's shared KV head end-to-end (QKV projection, RoPE,
causal attention, output projection) and writes a partial [T, HID] fp16
output; the host sums the 8 partials (the row-parallel all-reduce).

Precision plan (gate is rel-err < 2e-2; this lands ~1.4e-2):
  - QKV + output projections: fp16 x fp16 matmuls (1 PE cycle/row).
  - Scores: fp8 DoubleRow, one instruction per key tile computing
    k8^T(q_hi + q_lo) -- the two DoubleRow subtile slots carry a hi/lo fp8
    split of q, so the q side is ~exact and only k carries fp8 noise.
    0.5 cycles/row: 2x fp32r.
  - exp: ACT engine writes fp8e4 directly; softmax numerator/denominator use
    the SAME quantized e so weight-quantization largely cancels.
  - PV: two DoubleRow instructions per key-tile PAIR: (v_hi[2j],v_hi[2j+1])
    and (v_lo[...]) against the e pair -- v is hi/lo-split (~exact), e noise
    cancels through the denominator. 2x fp32r.
  - Denominator: DoubleRow over e pairs with a ones stationary: 4x fp32r.

Schedule: one fused software pipeline. Block 0's QKV runs first; thereafter
attention for block i runs with the QKV matmuls of block i+1 and the
output-projection units of block i-1 woven between its score/PV steps as
tensor-engine filler, so the PE stays dense (and at full p-state clock)
while ACT works through the exps. All of block i+1's hidden-stream DMAs are
prefetched at the start of attention i; RoPE for block i+1 is emitted as
soon as its last hc lands so its latency hides under block i's remaining
pairs. Output-projection units rotate over four PSUM banks (the scratch
bank plus the three QKV banks, which are idle between accumulations).

Layouts on device (partition dim first):
  hT16    [HID, T] fp16 (host-transposed)  -> moving operand of QKV matmuls
  q8      [D, 2(hi,lo), T] fp8 per head    -> scores moving
  k8      [D, 2(dup), T] fp8               -> scores stationary slices
  v hi/lo [T-part, kt, D] fp8 natural      -> PV stationary; produced by
          per-token-tile matmuls (stationary = hT slice) -- no PE transpose
  e8      [kt, 2, TB] fp8 pair tiles       -> PV/denominator moving
  ctxT    [D, T] fp16                      -> output projection stationary
"""

import numpy as np
import ml_dtypes

import concourse.bass as bass
import concourse.mybir as mybir
import concourse.tile as tile
from concourse import bacc, bass_utils
from concourse.bass import ts

F32 = mybir.dt.float32
F16 = mybir.dt.float16
F8 = mybir.dt.float8e4
AF = mybir.ActivationFunctionType
OP = mybir.AluOpType
DR = mybir.MatmulPerfMode.DoubleRow

H, KV, D, HID, T = 16, 4, 128, 2048, 2048
THETA = 10000.0
N_CORES = 8
QH = H // N_CORES            # query heads per core = 2
TB = 512                     # token block
NTB = T // TB                # 4
HCN = HID // 128             # 16 h-chunks
SCALE = float(D) ** -0.5
F8NP = ml_dtypes.float8_e4m3

# hc group layout: block 0 ramps up (small first DMA so the first matmul
# starts early); other blocks use 4-chunk groups
GROUPS0 = [(0, 1), (1, 3), (4, 4), (8, 4), (12, 4)]
GROUPS = [(0, 4), (4, 4), (8, 4), (12, 4)]


def _build():
    nc = bacc.Bacc("TRN2", target_bir_lowering=False, debug=False,
                   num_devices=N_CORES)

    hT_d = nc.dram_tensor("hT16", [HID, T], F16, kind="ExternalInput").ap()
    w_d = nc.dram_tensor("w16", [128, HCN, 4 * 128], F16, kind="ExternalInput").ap()
    cs_d = nc.dram_tensor("cs16", [128, 2, T], F16, kind="ExternalInput").ap()
    mask_d = nc.dram_tensor("mask16", [128, 4, TB], F16, kind="ExternalInput").ap()
    ones_d = nc.dram_tensor("ones8", [128, 2, 128], F8, kind="ExternalInput").ap()
    wo_d = nc.dram_tensor("wo16", [128, 2, HID], F16, kind="ExternalInput").ap()
    out_d = nc.dram_tensor("out16", [T, HID], F16, kind="ExternalOutput").ap()

    hT_view = hT_d.rearrange("(hc p) t -> hc p t", p=128)

    with tile.TileContext(nc) as tc:
        with tc.tile_pool(name="const", bufs=1) as cpool, \
             tc.tile_pool(name="acts", bufs=1) as apool, \
             tc.tile_pool(name="hstream", bufs=8) as hpool, \
             tc.tile_pool(name="rtmp", bufs=3) as tpool, \
             tc.tile_pool(name="p2e", bufs=8) as epool, \
             tc.tile_pool(name="p2tmp", bufs=2) as t2pool, \
             tc.tile_pool(name="p3out", bufs=2) as opool, \
             tc.tile_pool(name="qkvps", bufs=1, space="PSUM") as qkvps, \
             tc.tile_pool(name="scrps", bufs=1, space="PSUM") as scrps, \
             tc.tile_pool(name="sps", bufs=1, space="PSUM") as sps_pool, \
             tc.tile_pool(name="ops", bufs=1, space="PSUM") as ops_pool, \
             tc.tile_pool(name="cps", bufs=1, space="PSUM") as cps_pool, \
             tc.tile_pool(name="dps", bufs=1, space="PSUM") as dps_pool:

            w_sb = cpool.tile([128, HCN, 4 * 128], F16)
            cs_sb = cpool.tile([128, 2, T], F16)
            mask_sb = cpool.tile([128, 4, TB], F16)
            ones_sb = cpool.tile([128, 2, 128], F8)
            wo_sb = cpool.tile([128, 2, HID], F16)

            q8 = [apool.tile([128, 2, T], F8, name=f"q8_{i}", tag=f"q8_{i}")
                  for i in range(QH)]
            k8 = apool.tile([128, 2, T], F8)
            vhi = apool.tile([128, 16, 128], F8)
            vlo = apool.tile([128, 16, 128], F8)
            ctxT = [apool.tile([128, T], F16, name=f"ctxT{i}", tag=f"ctxT{i}")
                    for i in range(QH)]

            # ---------------- emit helpers ----------------
            qkv_live = {}   # block -> [ps_q0, ps_q1, ps_k]
            vnat_live = {}  # block -> ps_v (from scratch pool)
            h_tiles = {}    # (block, group-idx) -> sbuf tile
            h_keep = {}     # retained h tiles for deferred v-nat

            def emit_h_dma(i, gi):
                hc0, wid = (GROUPS0 if i == 0 else GROUPS)[gi]
                hT_t = hpool.tile([128, wid, TB], F16, tag=f"h{wid}")
                src = hT_view[hc0:hc0 + wid, :, ts(i, TB)]
                nc.sync.dma_start(hT_t[:], src.rearrange("g p t -> p g t"))
                h_tiles[(i, gi)] = hT_t

            def emit_qkv_group(i, gi):
                hc0, wid = (GROUPS0 if i == 0 else GROUPS)[gi]
                if gi == 0:
                    qkv_live[i] = [qkvps.tile([128, TB], F32, name=f"psqkv{n}",
                                              tag=f"qkv{n}") for n in range(3)]
                ps_qkv = qkv_live[i]
                hT_t = h_tiles.pop((i, gi))
                h_keep[(i, gi)] = hT_t
                for j in range(wid):
                    hc = hc0 + j
                    for n in range(3):
                        nc.tensor.matmul(ps_qkv[n][:], w_sb[:, hc, ts(n, 128)],
                                         hT_t[:, j, :], start=(hc == 0),
                                         stop=(hc == HCN - 1))

            def emit_vnat(i):
                # v in natural layout from the retained h tiles. tt-OUTER:
                # each PSUM sub-region's accumulation group must run
                # start->stop consecutively (interleaved same-bank
                # accumulation groups produce garbage on hardware).
                vnat_live[i] = scrps.tile([128, 4, 128], F32, name="scr",
                                          tag="scr")
                ps_v = vnat_live[i]
                groups = GROUPS0 if i == 0 else GROUPS
                for tt in range(4):
                    for gi, (hc0, wid) in enumerate(groups):
                        hT_t = h_keep[(i, gi)]
                        for j in range(wid):
                            hc = hc0 + j
                            nc.tensor.matmul(ps_v[:, tt, :],
                                             hT_t[:, j, ts(tt, 128)],
                                             w_sb[:, hc, ts(3, 128)],
                                             start=(hc == 0),
                                             stop=(hc == HCN - 1))
                for gi in range(len(groups)):
                    h_keep.pop((i, gi))
                # v hi/lo casts
                nc.vector.tensor_copy(vhi[:, ts(i, 4), :], ps_v[:])
                nc.vector.tensor_tensor(out=vlo[:, ts(i, 4), :], in0=ps_v[:],
                                        in1=vhi[:, ts(i, 4), :], op=OP.subtract)
                vnat_live.pop(i)

            def emit_rope(i):
                ps_qkv = qkv_live.pop(i)
                # fp16 RoPE; k chunk (slot 2) first so scores can start
                # after k + q0 land
                x_all = tpool.tile([128, 3, TB], F16, tag="ropex")
                for n in (2, 0, 1):
                    nc.vector.tensor_copy(x_all[:, n, :], ps_qkv[n][:])
                xsw = tpool.tile([128, 3, TB], F16, tag="ropesw")
                nc.scalar.dma_start(xsw[0:64, :, :], x_all[64:128, :, :])
                nc.scalar.dma_start(xsw[64:128, :, :], x_all[0:64, :, :])
                for n in (2, 0, 1):
                    t2 = tpool.tile([128, TB], F16, tag="ropet2")
                    nc.vector.tensor_tensor(out=t2[:], in0=xsw[:, n, :],
                                            in1=cs_sb[:, 1, ts(i, TB)],
                                            op=OP.mult)
                    m1 = tpool.tile([128, TB], F16, tag="ropem1")
                    nc.gpsimd.tensor_tensor(out=m1[:], in0=x_all[:, n, :],
                                            in1=cs_sb[:, 0, ts(i, TB)],
                                            op=OP.mult)
                    if n < QH:
                        qf = tpool.tile([128, TB], F16, tag="ropeqf")
                        nc.vector.tensor_tensor(out=qf[:], in0=m1[:], in1=t2[:],
                                                op=OP.add)
                        nc.gpsimd.tensor_copy(q8[n][:, 0, ts(i, TB)], qf[:])
                        nc.vector.tensor_tensor(
                            out=q8[n][:, 1, ts(i, TB)], in0=qf[:],
                            in1=q8[n][:, 0, ts(i, TB)], op=OP.subtract)
                    else:
                        nc.vector.tensor_tensor(out=k8[:, 0, ts(i, TB)],
                                                in0=m1[:], in1=t2[:], op=OP.add)
                        nc.scalar.copy(k8[:, 1, ts(i, TB)], k8[:, 0, ts(i, TB)])

            pending = []    # outproj units (tt, n) awaiting emission
            osb_map = {}
            unit_ctr = [0]
            TAIL_TAGS = ["ops", "qkv0", "qkv1", "qkv2"]

            def emit_outproj_unit(tail=False, dve_only=False):
                tt, n = pending.pop(0)
                if tt not in osb_map:
                    osb_map[tt] = opool.tile([128, HID], F16, name="o_sb")
                o_sb = osb_map[tt]
                unit_ctr[0] += 1
                tag = TAIL_TAGS[unit_ctr[0] % 4] if tail else "ops"
                if tag == "ops":
                    ps_o = ops_pool.tile([128, TB], F32, name="ps_o", tag="ops")
                else:
                    ps_o = qkvps.tile([128, TB], F32, name=f"ps{tag}", tag=tag)
                nc.tensor.matmul(ps_o[:], ctxT[0][:, ts(tt, 128)],
                                 wo_sb[:, 0, ts(n, 512)], start=True, stop=False)
                nc.tensor.matmul(ps_o[:], ctxT[1][:, ts(tt, 128)],
                                 wo_sb[:, 1, ts(n, 512)], start=False, stop=True)
                if dve_only or unit_ctr[0] % 2 == 0:
                    nc.vector.tensor_copy(o_sb[:, ts(n, 512)], ps_o[:])
                else:
                    nc.scalar.copy(o_sb[:, ts(n, 512)], ps_o[:])
                if n == 3:
                    nc.sync.dma_start(out_d[ts(tt, 128), :], o_sb[:])
                    del osb_map[tt]

            def emit_attn_block(i):
                """Attention for block i, weaving in QKV matmuls of block
                i+1 and outproj units of block i-1 as PE filler."""
                ngrp = len(GROUPS) if i < NTB - 1 else 0
                grp_q = list(range(ngrp))
                if i < NTB - 1:
                    for gi in range(ngrp):
                        emit_h_dma(i + 1, gi)
                    nc.sync.dma_start(cs_sb[:, :, ts(i + 1, TB)],
                                      cs_d[:, :, ts(i + 1, TB)])
                if i == 0:
                    nc.sync.dma_start(mask_sb[:], mask_d)
                    nc.sync.dma_start(ones_sb[:], ones_d)
                    nc.sync.dma_start(wo_sb[:], wo_d)
                if i == 0:
                    # block 0's v-nat runs here, out of its DMA-bound QKV
                    # stream, covering RoPE(0)'s latency tail
                    emit_vnat(0)
                npair = 2 * (i + 1)
                steps_total = npair * QH
                step = [0]

                def filler():
                    steps_left = steps_total - step[0]
                    step[0] += 1
                    if grp_q:
                        for _ in range(2 if i <= 1 else 1):
                            if grp_q:
                                emit_qkv_group(i + 1, grp_q.pop(0))
                        if not grp_q:
                            emit_vnat(i + 1)
                            emit_rope(i + 1)
                        return
                    if pending and steps_left > 0:
                        n_fill = (len(pending) + steps_left - 1) // steps_left
                        for _ in range(min(n_fill, len(pending))):
                            emit_outproj_unit(tail=(i == NTB - 1),
                                              dve_only=(i == NTB - 1))

                for qh in range(QH):
                    ctx_ps = cps_pool.tile([128, TB], F32, name="ctx_ps")
                    den_ps = dps_pool.tile([128, TB], F32, name="den_ps")
                    nkt = 4 * (i + 1)
                    e_tiles = [None] * npair

                    def emit_score_kt(kt, i=i, qh=qh, e_tiles=e_tiles):
                        pj, si = kt // 2, kt % 2
                        if si == 0:
                            e_tiles[pj] = epool.tile([128, 2, TB], F8,
                                                     name="e8", tag="e8")
                        e8t = e_tiles[pj]
                        if i == NTB - 1:
                            stag = ["s_ps", "qkv0", "qkv1", "qkv2"][kt % 4]
                        else:
                            stag = "s_ps"
                        if stag == "s_ps":
                            s_ps = sps_pool.tile([128, TB], F32, name="s_ps")
                        else:
                            s_ps = qkvps.tile([128, TB], F32, name=stag,
                                              tag=stag)
                        j = kt - 4 * i
                        lo = 128 * j if j > 0 else 0
                        nc.tensor.matmul(s_ps[:, lo:TB], k8[:, :, ts(kt, 128)],
                                         q8[qh][:, :, i * TB + lo:
                                                (i + 1) * TB],
                                         start=True, stop=True, perf_mode=DR)
                        nc.scalar.activation(e8t[:, si, lo:TB],
                                             s_ps[:, lo:TB], AF.Exp,
                                             scale=SCALE)
                        if j >= 0:
                            if lo > 0:
                                # the skipped prefix holds stale pool bytes
                                # (can be fp8 NaN/Inf -- x*0 would keep NaN):
                                # zero it explicitly on the idle Pool engine
                                nc.gpsimd.memset(e8t[:, si, 0:lo], 0)
                            nc.vector.tensor_tensor(
                                out=e8t[:, si, lo:lo + 128],
                                in0=e8t[:, si, lo:lo + 128],
                                in1=mask_sb[:, j, lo:lo + 128], op=OP.mult)

                    def emit_consume_piece(ck, npair=npair, ctx_ps=ctx_ps,
                                           den_ps=den_ps, e_tiles=e_tiles):
                        pj = ck // 2
                        e8t = e_tiles[pj]
                        if ck % 2 == 0:
                            nc.tensor.matmul(ctx_ps[:], vhi[:, ts(pj, 2), :],
                                             e8t[:], start=(pj == 0),
                                             stop=False, perf_mode=DR)
                        else:
                            nc.tensor.matmul(ctx_ps[:], vlo[:, ts(pj, 2), :],
                                             e8t[:], start=False,
                                             stop=(pj == npair - 1),
                                             perf_mode=DR)
                            nc.tensor.matmul(den_ps[:], ones_sb[:], e8t[:],
                                             start=(pj == 0),
                                             stop=(pj == npair - 1),
                                             perf_mode=DR)

                    LAG = 3
                    for k in range(nkt + LAG):
                        if k < nkt:
                            emit_score_kt(k)
                        ck = k - LAG
                        if ck >= 0:
                            emit_consume_piece(ck)
                            if ck % 2 == 1:
                                filler()

                    recip = t2pool.tile([128, TB], F32, tag="recip",
                                        name="recip")
                    nc.vector.reciprocal(recip[:], den_ps[:])
                    nc.vector.tensor_tensor(out=ctxT[qh][:, ts(i, TB)],
                                            in0=ctx_ps[:], in1=recip[:],
                                            op=OP.mult)

            # ---------------- main pipeline ----------------
            emit_h_dma(0, 0)
            nc.sync.dma_start(w_sb[:, 0:1, :], w_d[:, 0:1, :])
            emit_h_dma(0, 1)
            nc.sync.dma_start(w_sb[:, 1:4, :], w_d[:, 1:4, :])
            nc.sync.dma_start(cs_sb[:, :, ts(0, TB)], cs_d[:, :, ts(0, TB)])
            emit_h_dma(0, 2)
            nc.sync.dma_start(w_sb[:, 4:8, :], w_d[:, 4:8, :])
            emit_h_dma(0, 3)
            nc.sync.dma_start(w_sb[:, 8:12, :], w_d[:, 8:12, :])
            emit_h_dma(0, 4)
            nc.sync.dma_start(w_sb[:, 12:16, :], w_d[:, 12:16, :])
            for gi in range(len(GROUPS0)):
                emit_qkv_group(0, gi)
            emit_rope(0)
            for i in range(NTB):
                emit_attn_block(i)
                pending.extend((tt, n) for tt in range(4 * i, 4 * i + 4)
                               for n in range(4))
            while pending:
                emit_outproj_unit(tail=True)

    nc.compile()
    return nc


_NC_CACHE = None


def _get_nc():
    global _NC_CACHE
    if _NC_CACHE is None:
        _NC_CACHE = _build()
    return _NC_CACHE


def _host_tables(position_ids: np.ndarray):
    pos = np.asarray(position_ids, np.float32)
    inv_freq = (1.0 / (THETA ** (np.arange(0, D, 2, dtype=np.float32) / D)))
    ang = pos[:, None] * inv_freq[None, :]          # [T, 64] f32
    cos = np.cos(ang).T                             # [64, T]
    sin = np.sin(ang).T
    cosT = np.concatenate([cos, cos], axis=0).astype(np.float16)
    sinT = np.concatenate([-sin, sin], axis=0).astype(np.float16)
    return cosT, sinT


def _host_masks():
    # mask for diagonal tile j (keys 128j..128j+128 of the block): columns
    # [0, 128(j+1)): zero where q < k, i.e. col < 128j + row
    r = np.arange(128)[:, None]
    c = np.arange(TB)[None, :]
    m = np.stack([(c - r - 128 * j >= 0) for j in range(4)], axis=1)
    return m.astype(np.float16)                     # [128, 4, TB]


def kernel(hidden_states, position_ids, Wqkv, Wo):
    hidden_states = np.asarray(hidden_states, np.float32)
    Wqkv = np.asarray(Wqkv, np.float32)
    Wo = np.asarray(Wo, np.float32)

    nc = _get_nc()

    hT16 = np.ascontiguousarray(hidden_states.T).astype(np.float16)
    cosT, sinT = _host_tables(position_ids)
    cs16 = np.ascontiguousarray(np.stack([cosT, sinT], axis=1))  # [128,2,T]
    masks = _host_masks()
    ones8 = np.ones((128, 2, 128), dtype=F8NP)

    wq = Wqkv[:, : H * D]
    wk = Wqkv[:, H * D: (H + KV) * D]
    wv = Wqkv[:, (H + KV) * D:]

    in_maps = []
    for c in range(N_CORES):
        kvh = (c * QH) // (H // KV)
        w_cols = np.concatenate(
            [wq[:, (c * QH) * D: (c * QH + 1) * D],
             wq[:, (c * QH + 1) * D: (c * QH + 2) * D],
             wk[:, kvh * D: (kvh + 1) * D],
             wv[:, kvh * D: (kvh + 1) * D]], axis=1)         # [HID, 512]
        w16 = np.ascontiguousarray(
            w_cols.reshape(HCN, 128, 4 * 128).transpose(1, 0, 2)
        ).astype(np.float16)                                 # [128, HCN, 512]
        wo_local = Wo[c * QH * D: (c + 1) * QH * D, :]       # [256, HID]
        wo16 = np.ascontiguousarray(
            wo_local.reshape(2, 128, HID).transpose(1, 0, 2)
        ).astype(np.float16)                                 # [128, 2, HID]
        in_maps.append({
            "hT16": hT16, "w16": w16, "cs16": cs16,
            "mask16": masks, "ones8": ones8, "wo16": wo16,
        })

    res = bass_utils.run_bass_kernel_spmd(nc, in_maps,
                                          core_ids=list(range(N_CORES)))
    parts = np.stack([res.results[c]["out16"].astype(np.float32)
                      for c in range(N_CORES)], 0)
    return parts.sum(axis=0, dtype=np.float32)


# revision 43
# speedup vs baseline: 1.3329x; 1.0208x over previous
"""Bass/Trainium2 kernel for BailingAttention (GQA prefill, causal, RoPE).

Sharding: tensor-parallel over heads across 8 NeuronCores. Each core computes
2 query heads + its group's shared KV head end-to-end (QKV projection, RoPE,
causal attention, output projection) and writes a partial [T, HID] fp16
output; the host sums the 8 partials (the row-parallel all-reduce).

Precision plan (gate is rel-err < 2e-2; this lands ~1.4e-2):
  - QKV + output projections: fp16 x fp16 matmuls (1 PE cycle/row).
  - Scores: fp8 DoubleRow, one instruction per key tile computing
    k8^T(q_hi + q_lo) -- the two DoubleRow subtile slots carry a hi/lo fp8
    split of q, so the q side is ~exact and only k carries fp8 noise.
    0.5 cycles/row: 2x fp32r.
  - exp: ACT engine writes fp8e4 directly; softmax numerator/denominator use
    the SAME quantized e so weight-quantization largely cancels.
  - PV: two DoubleRow instructions per key-tile PAIR: (v_hi[2j],v_hi[2j+1])
    and (v_lo[...]) against the e pair -- v is hi/lo-split (~exact), e noise
    cancels through the denominator. 2x fp32r.
  - Denominator: DoubleRow over e pairs with a ones stationary: 4x fp32r.

Schedule: one fused software pipeline. Block 0's QKV runs first; thereafter
attention for block i runs with the QKV matmuls of block i+1 and the
output-projection units of block i-1 woven between its score/PV steps as
tensor-engine filler, so the PE stays dense (and at full p-state clock)
while ACT works through the exps. All of block i+1's hidden-stream DMAs are
prefetched at the start of attention i; RoPE for block i+1 is emitted as
soon as its last hc lands so its latency hides under block i's remaining
pairs. Output-projection units rotate over four PSUM banks (the scratch
bank plus the three QKV banks, which are idle between accumulations).

Layouts on device (partition dim first):
  hT16    [HID, T] fp16 (host-transposed)  -> moving operand of QKV matmuls
  q8      [D, 2(hi,lo), T] fp8 per head    -> scores moving
  k8      [D, 2(dup), T] fp8               -> scores stationary slices
  v hi/lo [T-part, kt, D] fp8 natural      -> PV stationary; produced by
          per-token-tile matmuls (stationary = hT slice) -- no PE transpose
  e8      [kt, 2, TB] fp8 pair tiles       -> PV/denominator moving
  ctxT    [D, T] fp16                      -> output projection stationary
"""

import numpy as np
import ml_dtypes

import concourse.bass as bass
import concourse.mybir as mybir
import concourse.tile as tile
from concourse import bacc, bass_utils
from concourse.bass import ts

F32 = mybir.dt.float32
F16 = mybir.dt.float16
F8 = mybir.dt.float8e4
AF = mybir.ActivationFunctionType
OP = mybir.AluOpType
DR = mybir.MatmulPerfMode.DoubleRow

H, KV, D, HID, T = 16, 4, 128, 2048, 2048
THETA = 10000.0
N_CORES = 8
QH = H // N_CORES            # query heads per core = 2
TB = 512                     # token block
NTB = T // TB                # 4
HCN = HID // 128             # 16 h-chunks
SCALE = float(D) ** -0.5
F8NP = ml_dtypes.float8_e4m3

# hc group layout: block 0 ramps up (small first DMA so the first matmul
# starts early); other blocks use 4-chunk groups
GROUPS0 = [(0, 1), (1, 3), (4, 4), (8, 4), (12, 4)]
GROUPS = [(0, 4), (4, 4), (8, 4), (12, 4)]


def _build():
    nc = bacc.Bacc("TRN2", target_bir_lowering=False, debug=False,
                   num_devices=N_CORES)

    hT_d = nc.dram_tensor("hT16", [HID, T], F16, kind="ExternalInput").ap()
    w_d = nc.dram_tensor("w16", [128, HCN, 4 * 128], F16, kind="ExternalInput").ap()
    cs_d = nc.dram_tensor("cs16", [128, 2, T], F16, kind="ExternalInput").ap()
    mask_d = nc.dram_tensor("mask16", [128, 4, TB], F16, kind="ExternalInput").ap()
    ones_d = nc.dram_tensor("ones8", [128, 2, 128], F8, kind="ExternalInput").ap()
    wo_d = nc.dram_tensor("wo16", [128, 2, HID], F16, kind="ExternalInput").ap()
    out_d = nc.dram_tensor("out16", [T, HID], F16, kind="ExternalOutput").ap()

    hT_view = hT_d.rearrange("(hc p) t -> hc p t", p=128)

    with tile.TileContext(nc) as tc:
        with tc.tile_pool(name="const", bufs=1) as cpool, \
             tc.tile_pool(name="acts", bufs=1) as apool, \
             tc.tile_pool(name="hstream", bufs=8) as hpool, \
             tc.tile_pool(name="rtmp", bufs=3) as tpool, \
             tc.tile_pool(name="p2e", bufs=8) as epool, \
             tc.tile_pool(name="p2tmp", bufs=2) as t2pool, \
             tc.tile_pool(name="p3out", bufs=2) as opool, \
             tc.tile_pool(name="qkvps", bufs=1, space="PSUM") as qkvps, \
             tc.tile_pool(name="scrps", bufs=1, space="PSUM") as scrps, \
             tc.tile_pool(name="sps", bufs=1, space="PSUM") as sps_pool, \
             tc.tile_pool(name="ops", bufs=1, space="PSUM") as ops_pool, \
             tc.tile_pool(name="cps", bufs=1, space="PSUM") as cps_pool, \
             tc.tile_pool(name="dps", bufs=1, space="PSUM") as dps_pool:

            w_sb = cpool.tile([128, HCN, 4 * 128], F16)
            cs_sb = cpool.tile([128, 2, T], F16)
            mask_sb = cpool.tile([128, 4, TB], F16)
            ones_sb = cpool.tile([128, 2, 128], F8)
            wo_sb = cpool.tile([128, 2, HID], F16)

            q8 = [apool.tile([128, 2, T], F8, name=f"q8_{i}", tag=f"q8_{i}")
                  for i in range(QH)]
            k8 = apool.tile([128, 2, T], F8)
            vhi = apool.tile([128, 16, 128], F8)
            vlo = apool.tile([128, 16, 128], F8)
            ctxT = [apool.tile([128, T], F16, name=f"ctxT{i}", tag=f"ctxT{i}")
                    for i in range(QH)]

            # ---------------- emit helpers ----------------
            qkv_live = {}   # block -> [ps_q0, ps_q1, ps_k]
            vnat_live = {}  # block -> ps_v (from scratch pool)
            h_tiles = {}    # (block, group-idx) -> sbuf tile
            h_keep = {}     # retained h tiles for deferred v-nat

            def emit_h_dma(i, gi):
                hc0, wid = (GROUPS0 if i == 0 else GROUPS)[gi]
                hT_t = hpool.tile([128, wid, TB], F16, tag=f"h{wid}")
                src = hT_view[hc0:hc0 + wid, :, ts(i, TB)]
                nc.sync.dma_start(hT_t[:], src.rearrange("g p t -> p g t"))
                h_tiles[(i, gi)] = hT_t

            def emit_qkv_group(i, gi):
                hc0, wid = (GROUPS0 if i == 0 else GROUPS)[gi]
                if gi == 0:
                    qkv_live[i] = [qkvps.tile([128, TB], F32, name=f"psqkv{n}",
                                              tag=f"qkv{n}") for n in range(3)]
                ps_qkv = qkv_live[i]
                hT_t = h_tiles.pop((i, gi))
                h_keep[(i, gi)] = hT_t
                for j in range(wid):
                    hc = hc0 + j
                    for n in range(3):
                        nc.tensor.matmul(ps_qkv[n][:], w_sb[:, hc, ts(n, 128)],
                                         hT_t[:, j, :], start=(hc == 0),
                                         stop=(hc == HCN - 1))

            def emit_vnat(i):
                # v in natural layout from the retained h tiles. tt-OUTER:
                # each PSUM sub-region's accumulation group must run
                # start->stop consecutively (interleaved same-bank
                # accumulation groups produce garbage on hardware).
                vnat_live[i] = scrps.tile([128, 4, 128], F32, name="scr",
                                          tag="scr")
                ps_v = vnat_live[i]
                groups = GROUPS0 if i == 0 else GROUPS
                for tt in range(4):
                    for gi, (hc0, wid) in enumerate(groups):
                        hT_t = h_keep[(i, gi)]
                        for j in range(wid):
                            hc = hc0 + j
                            nc.tensor.matmul(ps_v[:, tt, :],
                                             hT_t[:, j, ts(tt, 128)],
                                             w_sb[:, hc, ts(3, 128)],
                                             start=(hc == 0),
                                             stop=(hc == HCN - 1))
                for gi in range(len(groups)):
                    h_keep.pop((i, gi))
                # v hi/lo casts
                nc.vector.tensor_copy(vhi[:, ts(i, 4), :], ps_v[:])
                nc.vector.tensor_tensor(out=vlo[:, ts(i, 4), :], in0=ps_v[:],
                                        in1=vhi[:, ts(i, 4), :], op=OP.subtract)
                vnat_live.pop(i)

            def emit_rope(i):
                ps_qkv = qkv_live.pop(i)
                # fp16 RoPE; k chunk (slot 2) first so scores can start
                # after k + q0 land
                x_all = tpool.tile([128, 3, TB], F16, tag="ropex")
                for n in (2, 0, 1):
                    nc.vector.tensor_copy(x_all[:, n, :], ps_qkv[n][:])
                xsw = tpool.tile([128, 3, TB], F16, tag="ropesw")
                nc.scalar.dma_start(xsw[0:64, :, :], x_all[64:128, :, :])
                nc.scalar.dma_start(xsw[64:128, :, :], x_all[0:64, :, :])
                for n in (2, 0, 1):
                    t2 = tpool.tile([128, TB], F16, tag="ropet2")
                    nc.vector.tensor_tensor(out=t2[:], in0=xsw[:, n, :],
                                            in1=cs_sb[:, 1, ts(i, TB)],
                                            op=OP.mult)
                    m1 = tpool.tile([128, TB], F16, tag="ropem1")
                    nc.gpsimd.tensor_tensor(out=m1[:], in0=x_all[:, n, :],
                                            in1=cs_sb[:, 0, ts(i, TB)],
                                            op=OP.mult)
                    if n < QH:
                        qf = tpool.tile([128, TB], F16, tag="ropeqf")
                        nc.vector.tensor_tensor(out=qf[:], in0=m1[:], in1=t2[:],
                                                op=OP.add)
                        nc.gpsimd.tensor_copy(q8[n][:, 0, ts(i, TB)], qf[:])
                        nc.vector.tensor_tensor(
                            out=q8[n][:, 1, ts(i, TB)], in0=qf[:],
                            in1=q8[n][:, 0, ts(i, TB)], op=OP.subtract)
                    else:
                        nc.vector.tensor_tensor(out=k8[:, 0, ts(i, TB)],
                                                in0=m1[:], in1=t2[:], op=OP.add)
                        nc.scalar.copy(k8[:, 1, ts(i, TB)], k8[:, 0, ts(i, TB)])

            pending = []    # outproj units (tt, n) awaiting emission
            osb_map = {}
            unit_ctr = [0]
            TAIL_TAGS = ["ops", "qkv0", "qkv1", "qkv2"]

            def emit_outproj_unit(tail=False, dve_only=False):
                tt, n = pending.pop(0)
                if tt not in osb_map:
                    osb_map[tt] = opool.tile([128, HID], F16, name="o_sb")
                o_sb = osb_map[tt]
                unit_ctr[0] += 1
                if tail:
                    tag = TAIL_TAGS[unit_ctr[0] % 4]
                else:
                    tag = ["ops", "scr"][unit_ctr[0] % 2]
                if tag == "ops":
                    ps_o = ops_pool.tile([128, TB], F32, name="ps_o", tag="ops")
                elif tag == "scr":
                    ps_o = scrps.tile([128, 4, 128], F32, name="scr", tag="scr")
                else:
                    ps_o = qkvps.tile([128, TB], F32, name=f"ps{tag}", tag=tag)
                nc.tensor.matmul(ps_o[:], ctxT[0][:, ts(tt, 128)],
                                 wo_sb[:, 0, ts(n, 512)], start=True, stop=False)
                nc.tensor.matmul(ps_o[:], ctxT[1][:, ts(tt, 128)],
                                 wo_sb[:, 1, ts(n, 512)], start=False, stop=True)
                if dve_only or unit_ctr[0] % 2 == 0:
                    nc.vector.tensor_copy(o_sb[:, ts(n, 512)], ps_o[:])
                else:
                    nc.scalar.copy(o_sb[:, ts(n, 512)], ps_o[:])
                if n == 3:
                    nc.sync.dma_start(out_d[ts(tt, 128), :], o_sb[:])
                    del osb_map[tt]

            def emit_attn_block(i):
                """Attention for block i, weaving in QKV matmuls of block
                i+1 and outproj units of block i-1 as PE filler."""
                ngrp = len(GROUPS) if i < NTB - 1 else 0
                grp_q = list(range(ngrp))
                if i < NTB - 1:
                    for gi in range(ngrp):
                        emit_h_dma(i + 1, gi)
                    nc.sync.dma_start(cs_sb[:, :, ts(i + 1, TB)],
                                      cs_d[:, :, ts(i + 1, TB)])
                if i == 0:
                    nc.sync.dma_start(mask_sb[:], mask_d)
                    nc.sync.dma_start(ones_sb[:], ones_d)
                    nc.sync.dma_start(wo_sb[:], wo_d)
                if i == 0:
                    # block 0's v-nat runs here, out of its DMA-bound QKV
                    # stream, covering RoPE(0)'s latency tail
                    emit_vnat(0)
                npair = 2 * (i + 1)
                steps_total = npair * QH
                step = [0]

                def filler():
                    steps_left = steps_total - step[0]
                    step[0] += 1
                    if grp_q:
                        for _ in range(2 if i <= 1 else 1):
                            if grp_q:
                                emit_qkv_group(i + 1, grp_q.pop(0))
                        if not grp_q:
                            emit_vnat(i + 1)
                            emit_rope(i + 1)
                        return
                    if pending and steps_left > 0:
                        n_fill = (len(pending) + steps_left - 1) // steps_left
                        for _ in range(min(n_fill, len(pending))):
                            emit_outproj_unit(tail=(i == NTB - 1),
                                              dve_only=(i == NTB - 1))

                for qh in range(QH):
                    ctx_ps = cps_pool.tile([128, TB], F32, name="ctx_ps")
                    den_ps = dps_pool.tile([128, TB], F32, name="den_ps")
                    nkt = 4 * (i + 1)
                    e_tiles = [None] * npair

                    def emit_score_kt(kt, i=i, qh=qh, e_tiles=e_tiles):
                        pj, si = kt // 2, kt % 2
                        if si == 0:
                            e_tiles[pj] = epool.tile([128, 2, TB], F8,
                                                     name="e8", tag="e8")
                        e8t = e_tiles[pj]
                        if i == NTB - 1:
                            stag = ["s_ps", "qkv0", "qkv1", "qkv2"][kt % 4]
                        else:
                            stag = "s_ps"
                        if stag == "s_ps":
                            s_ps = sps_pool.tile([128, TB], F32, name="s_ps")
                        else:
                            s_ps = qkvps.tile([128, TB], F32, name=stag,
                                              tag=stag)
                        j = kt - 4 * i
                        lo = 128 * j if j > 0 else 0
                        nc.tensor.matmul(s_ps[:, lo:TB], k8[:, :, ts(kt, 128)],
                                         q8[qh][:, :, i * TB + lo:
                                                (i + 1) * TB],
                                         start=True, stop=True, perf_mode=DR)
                        nc.scalar.activation(e8t[:, si, lo:TB],
                                             s_ps[:, lo:TB], AF.Exp,
                                             scale=SCALE)
                        if j >= 0:
                            if lo > 0:
                                # the skipped prefix holds stale pool bytes
                                # (can be fp8 NaN/Inf -- x*0 would keep NaN):
                                # zero it explicitly on the idle Pool engine
                                nc.gpsimd.memset(e8t[:, si, 0:lo], 0)
                            nc.vector.tensor_tensor(
                                out=e8t[:, si, lo:lo + 128],
                                in0=e8t[:, si, lo:lo + 128],
                                in1=mask_sb[:, j, lo:lo + 128], op=OP.mult)

                    def emit_consume_piece(ck, npair=npair, ctx_ps=ctx_ps,
                                           den_ps=den_ps, e_tiles=e_tiles):
                        pj = ck // 2
                        e8t = e_tiles[pj]
                        if ck % 2 == 0:
                            nc.tensor.matmul(ctx_ps[:], vhi[:, ts(pj, 2), :],
                                             e8t[:], start=(pj == 0),
                                             stop=False, perf_mode=DR)
                        else:
                            nc.tensor.matmul(ctx_ps[:], vlo[:, ts(pj, 2), :],
                                             e8t[:], start=False,
                                             stop=(pj == npair - 1),
                                             perf_mode=DR)
                            nc.tensor.matmul(den_ps[:], ones_sb[:], e8t[:],
                                             start=(pj == 0),
                                             stop=(pj == npair - 1),
                                             perf_mode=DR)

                    LAG = 3
                    for k in range(nkt + LAG):
                        if k < nkt:
                            emit_score_kt(k)
                        ck = k - LAG
                        if ck >= 0:
                            emit_consume_piece(ck)
                            if ck % 2 == 1:
                                filler()

                    recip = t2pool.tile([128, TB], F32, tag="recip",
                                        name="recip")
                    nc.vector.reciprocal(recip[:], den_ps[:])
                    nc.vector.tensor_tensor(out=ctxT[qh][:, ts(i, TB)],
                                            in0=ctx_ps[:], in1=recip[:],
                                            op=OP.mult)

            # ---------------- main pipeline ----------------
            emit_h_dma(0, 0)
            nc.sync.dma_start(w_sb[:, 0:1, :], w_d[:, 0:1, :])
            emit_h_dma(0, 1)
            nc.sync.dma_start(w_sb[:, 1:4, :], w_d[:, 1:4, :])
            nc.sync.dma_start(cs_sb[:, :, ts(0, TB)], cs_d[:, :, ts(0, TB)])
            emit_h_dma(0, 2)
            nc.sync.dma_start(w_sb[:, 4:8, :], w_d[:, 4:8, :])
            emit_h_dma(0, 3)
            nc.sync.dma_start(w_sb[:, 8:12, :], w_d[:, 8:12, :])
            emit_h_dma(0, 4)
            nc.sync.dma_start(w_sb[:, 12:16, :], w_d[:, 12:16, :])
            for gi in range(len(GROUPS0)):
                emit_qkv_group(0, gi)
            emit_rope(0)
            for i in range(NTB):
                emit_attn_block(i)
                pending.extend((tt, n) for tt in range(4 * i, 4 * i + 4)
                               for n in range(4))
            while pending:
                emit_outproj_unit(tail=True)

    nc.compile()
    return nc


_NC_CACHE = None


def _get_nc():
    global _NC_CACHE
    if _NC_CACHE is None:
        _NC_CACHE = _build()
    return _NC_CACHE


def _host_tables(position_ids: np.ndarray):
    pos = np.asarray(position_ids, np.float32)
    inv_freq = (1.0 / (THETA ** (np.arange(0, D, 2, dtype=np.float32) / D)))
    ang = pos[:, None] * inv_freq[None, :]          # [T, 64] f32
    cos = np.cos(ang).T                             # [64, T]
    sin = np.sin(ang).T
    cosT = np.concatenate([cos, cos], axis=0).astype(np.float16)
    sinT = np.concatenate([-sin, sin], axis=0).astype(np.float16)
    return cosT, sinT


def _host_masks():
    # mask for diagonal tile j (keys 128j..128j+128 of the block): columns
    # [0, 128(j+1)): zero where q < k, i.e. col < 128j + row
    r = np.arange(128)[:, None]
    c = np.arange(TB)[None, :]
    m = np.stack([(c - r - 128 * j >= 0) for j in range(4)], axis=1)
    return m.astype(np.float16)                     # [128, 4, TB]


def kernel(hidden_states, position_ids, Wqkv, Wo):
    hidden_states = np.asarray(hidden_states, np.float32)
    Wqkv = np.asarray(Wqkv, np.float32)
    Wo = np.asarray(Wo, np.float32)

    nc = _get_nc()

    hT16 = np.ascontiguousarray(hidden_states.T).astype(np.float16)
    cosT, sinT = _host_tables(position_ids)
    cs16 = np.ascontiguousarray(np.stack([cosT, sinT], axis=1))  # [128,2,T]
    masks = _host_masks()
    ones8 = np.ones((128, 2, 128), dtype=F8NP)

    wq = Wqkv[:, : H * D]
    wk = Wqkv[:, H * D: (H + KV) * D]
    wv = Wqkv[:, (H + KV) * D:]

    in_maps = []
    for c in range(N_CORES):
        kvh = (c * QH) // (H // KV)
        w_cols = np.concatenate(
            [wq[:, (c * QH) * D: (c * QH + 1) * D],
             wq[:, (c * QH + 1) * D: (c * QH + 2) * D],
             wk[:, kvh * D: (kvh + 1) * D],
             wv[:, kvh * D: (kvh + 1) * D]], axis=1)         # [HID, 512]
        w16 = np.ascontiguousarray(
            w_cols.reshape(HCN, 128, 4 * 128).transpose(1, 0, 2)
        ).astype(np.float16)                                 # [128, HCN, 512]
        wo_local = Wo[c * QH * D: (c + 1) * QH * D, :]       # [256, HID]
        wo16 = np.ascontiguousarray(
            wo_local.reshape(2, 128, HID).transpose(1, 0, 2)
        ).astype(np.float16)                                 # [128, 2, HID]
        in_maps.append({
            "hT16": hT16, "w16": w16, "cs16": cs16,
            "mask16": masks, "ones8": ones8, "wo16": wo16,
        })

    res = bass_utils.run_bass_kernel_spmd(nc, in_maps,
                                          core_ids=list(range(N_CORES)))
    parts = np.stack([res.results[c]["out16"].astype(np.float32)
                      for c in range(N_CORES)], 0)
    return parts.sum(axis=0, dtype=np.float32)


# revision 44
# speedup vs baseline: 1.3366x; 1.0027x over previous
"""Bass/Trainium2 kernel for BailingAttention (GQA prefill, causal, RoPE).

Sharding: tensor-parallel over heads across 8 NeuronCores. Each core computes
2 query heads + its group's shared KV head end-to-end (QKV projection, RoPE,
causal attention, output projection) and writes a partial [T, HID] fp16
output; the host sums the 8 partials (the row-parallel all-reduce).

Precision plan (gate is rel-err < 2e-2; this lands ~1.4e-2):
  - QKV + output projections: fp16 x fp16 matmuls (1 PE cycle/row).
  - Scores: fp8 DoubleRow, one instruction per key tile computing
    k8^T(q_hi + q_lo) -- the two DoubleRow subtile slots carry a hi/lo fp8
    split of q, so the q side is ~exact and only k carries fp8 noise.
    0.5 cycles/row: 2x fp32r.
  - exp: ACT engine writes fp8e4 directly; softmax numerator/denominator use
    the SAME quantized e so weight-quantization largely cancels.
  - PV: two DoubleRow instructions per key-tile PAIR: (v_hi[2j],v_hi[2j+1])
    and (v_lo[...]) against the e pair -- v is hi/lo-split (~exact), e noise
    cancels through the denominator. 2x fp32r.
  - Denominator: DoubleRow over e pairs with a ones stationary: 4x fp32r.

Schedule: one fused software pipeline. Block 0's QKV runs first; thereafter
attention for block i runs with the QKV matmuls of block i+1 and the
output-projection units of block i-1 woven between its score/PV steps as
tensor-engine filler, so the PE stays dense (and at full p-state clock)
while ACT works through the exps. All of block i+1's hidden-stream DMAs are
prefetched at the start of attention i; RoPE for block i+1 is emitted as
soon as its last hc lands so its latency hides under block i's remaining
pairs. Output-projection units rotate over four PSUM banks (the scratch
bank plus the three QKV banks, which are idle between accumulations).

Layouts on device (partition dim first):
  hT16    [HID, T] fp16 (host-transposed)  -> moving operand of QKV matmuls
  q8      [D, 2(hi,lo), T] fp8 per head    -> scores moving
  k8      [D, 2(dup), T] fp8               -> scores stationary slices
  v hi/lo [T-part, kt, D] fp8 natural      -> PV stationary; produced by
          per-token-tile matmuls (stationary = hT slice) -- no PE transpose
  e8      [kt, 2, TB] fp8 pair tiles       -> PV/denominator moving
  ctxT    [D, T] fp16                      -> output projection stationary
"""

import numpy as np
import ml_dtypes

import concourse.bass as bass
import concourse.mybir as mybir
import concourse.tile as tile
from concourse import bacc, bass_utils
from concourse.bass import ts

F32 = mybir.dt.float32
F16 = mybir.dt.float16
F8 = mybir.dt.float8e4
AF = mybir.ActivationFunctionType
OP = mybir.AluOpType
DR = mybir.MatmulPerfMode.DoubleRow

H, KV, D, HID, T = 16, 4, 128, 2048, 2048
THETA = 10000.0
N_CORES = 8
QH = H // N_CORES            # query heads per core = 2
TB = 512                     # token block
NTB = T // TB                # 4
HCN = HID // 128             # 16 h-chunks
SCALE = float(D) ** -0.5
F8NP = ml_dtypes.float8_e4m3

# hc group layout: block 0 ramps up (small first DMA so the first matmul
# starts early); other blocks use 4-chunk groups
GROUPS0 = [(0, 1), (1, 3), (4, 4), (8, 4), (12, 4)]
GROUPS = [(0, 4), (4, 4), (8, 4), (12, 4)]


def _build():
    nc = bacc.Bacc("TRN2", target_bir_lowering=False, debug=False,
                   num_devices=N_CORES)

    hT_d = nc.dram_tensor("hT16", [HID, T], F16, kind="ExternalInput").ap()
    w_d = nc.dram_tensor("w16", [128, HCN, 4 * 128], F16, kind="ExternalInput").ap()
    cs_d = nc.dram_tensor("cs16", [128, 2, T], F16, kind="ExternalInput").ap()
    mask_d = nc.dram_tensor("mask16", [128, 4, TB], F16, kind="ExternalInput").ap()
    ones_d = nc.dram_tensor("ones8", [128, 2, 128], F8, kind="ExternalInput").ap()
    wo_d = nc.dram_tensor("wo16", [128, 2, HID], F16, kind="ExternalInput").ap()
    out_d = nc.dram_tensor("out16", [T, HID], F16, kind="ExternalOutput").ap()

    hT_view = hT_d.rearrange("(hc p) t -> hc p t", p=128)

    with tile.TileContext(nc) as tc:
        with tc.tile_pool(name="const", bufs=1) as cpool, \
             tc.tile_pool(name="acts", bufs=1) as apool, \
             tc.tile_pool(name="hstream", bufs=8) as hpool, \
             tc.tile_pool(name="rtmp", bufs=3) as tpool, \
             tc.tile_pool(name="p2e", bufs=8) as epool, \
             tc.tile_pool(name="p2tmp", bufs=2) as t2pool, \
             tc.tile_pool(name="p3out", bufs=2) as opool, \
             tc.tile_pool(name="qkvps", bufs=1, space="PSUM") as qkvps, \
             tc.tile_pool(name="scrps", bufs=1, space="PSUM") as scrps, \
             tc.tile_pool(name="sps", bufs=1, space="PSUM") as sps_pool, \
             tc.tile_pool(name="ops", bufs=1, space="PSUM") as ops_pool, \
             tc.tile_pool(name="cps", bufs=1, space="PSUM") as cps_pool, \
             tc.tile_pool(name="dps", bufs=1, space="PSUM") as dps_pool:

            w_sb = cpool.tile([128, HCN, 4 * 128], F16)
            cs_sb = cpool.tile([128, 2, T], F16)
            mask_sb = cpool.tile([128, 4, TB], F16)
            ones_sb = cpool.tile([128, 2, 128], F8)
            wo_sb = cpool.tile([128, 2, HID], F16)

            q8 = [apool.tile([128, 2, T], F8, name=f"q8_{i}", tag=f"q8_{i}")
                  for i in range(QH)]
            k8 = apool.tile([128, 2, T], F8)
            vhi = apool.tile([128, 16, 128], F8)
            vlo = apool.tile([128, 16, 128], F8)
            ctxT = [apool.tile([128, T], F16, name=f"ctxT{i}", tag=f"ctxT{i}")
                    for i in range(QH)]

            # ---------------- emit helpers ----------------
            qkv_live = {}   # block -> [ps_q0, ps_q1, ps_k]
            vnat_live = {}  # block -> ps_v (from scratch pool)
            h_tiles = {}    # (block, group-idx) -> sbuf tile
            h_keep = {}     # retained h tiles for deferred v-nat

            def emit_h_dma(i, gi):
                hc0, wid = (GROUPS0 if i == 0 else GROUPS)[gi]
                hT_t = hpool.tile([128, wid, TB], F16, tag=f"h{wid}")
                src = hT_view[hc0:hc0 + wid, :, ts(i, TB)]
                nc.sync.dma_start(hT_t[:], src.rearrange("g p t -> p g t"))
                h_tiles[(i, gi)] = hT_t

            def emit_qkv_group(i, gi):
                hc0, wid = (GROUPS0 if i == 0 else GROUPS)[gi]
                if gi == 0:
                    qkv_live[i] = [qkvps.tile([128, TB], F32, name=f"psqkv{n}",
                                              tag=f"qkv{n}") for n in range(3)]
                ps_qkv = qkv_live[i]
                hT_t = h_tiles.pop((i, gi))
                h_keep[(i, gi)] = hT_t
                for j in range(wid):
                    hc = hc0 + j
                    for n in range(3):
                        nc.tensor.matmul(ps_qkv[n][:], w_sb[:, hc, ts(n, 128)],
                                         hT_t[:, j, :], start=(hc == 0),
                                         stop=(hc == HCN - 1))

            def emit_vnat(i):
                # v in natural layout from the retained h tiles. tt-OUTER:
                # each PSUM sub-region's accumulation group must run
                # start->stop consecutively (interleaved same-bank
                # accumulation groups produce garbage on hardware).
                vnat_live[i] = scrps.tile([128, 4, 128], F32, name="scr",
                                          tag="scr")
                ps_v = vnat_live[i]
                groups = GROUPS0 if i == 0 else GROUPS
                for tt in range(4):
                    for gi, (hc0, wid) in enumerate(groups):
                        hT_t = h_keep[(i, gi)]
                        for j in range(wid):
                            hc = hc0 + j
                            nc.tensor.matmul(ps_v[:, tt, :],
                                             hT_t[:, j, ts(tt, 128)],
                                             w_sb[:, hc, ts(3, 128)],
                                             start=(hc == 0),
                                             stop=(hc == HCN - 1))
                for gi in range(len(groups)):
                    h_keep.pop((i, gi))
                # v hi/lo casts
                nc.vector.tensor_copy(vhi[:, ts(i, 4), :], ps_v[:])
                nc.vector.tensor_tensor(out=vlo[:, ts(i, 4), :], in0=ps_v[:],
                                        in1=vhi[:, ts(i, 4), :], op=OP.subtract)
                vnat_live.pop(i)

            def emit_rope(i):
                ps_qkv = qkv_live.pop(i)
                # fp16 RoPE; k chunk (slot 2) first so scores can start
                # after k + q0 land
                x_all = tpool.tile([128, 3, TB], F16, tag="ropex")
                for n in (2, 0, 1):
                    nc.vector.tensor_copy(x_all[:, n, :], ps_qkv[n][:])
                xsw = tpool.tile([128, 3, TB], F16, tag="ropesw")
                nc.scalar.dma_start(xsw[0:64, :, :], x_all[64:128, :, :])
                nc.scalar.dma_start(xsw[64:128, :, :], x_all[0:64, :, :])
                for n in (2, 0, 1):
                    t2 = tpool.tile([128, TB], F16, tag="ropet2")
                    nc.vector.tensor_tensor(out=t2[:], in0=xsw[:, n, :],
                                            in1=cs_sb[:, 1, ts(i, TB)],
                                            op=OP.mult)
                    m1 = tpool.tile([128, TB], F16, tag="ropem1")
                    nc.gpsimd.tensor_tensor(out=m1[:], in0=x_all[:, n, :],
                                            in1=cs_sb[:, 0, ts(i, TB)],
                                            op=OP.mult)
                    if n < QH:
                        qf = tpool.tile([128, TB], F16, tag="ropeqf")
                        nc.vector.tensor_tensor(out=qf[:], in0=m1[:], in1=t2[:],
                                                op=OP.add)
                        nc.gpsimd.tensor_copy(q8[n][:, 0, ts(i, TB)], qf[:])
                        nc.vector.tensor_tensor(
                            out=q8[n][:, 1, ts(i, TB)], in0=qf[:],
                            in1=q8[n][:, 0, ts(i, TB)], op=OP.subtract)
                    else:
                        nc.vector.tensor_tensor(out=k8[:, 0, ts(i, TB)],
                                                in0=m1[:], in1=t2[:], op=OP.add)
                        nc.scalar.copy(k8[:, 1, ts(i, TB)], k8[:, 0, ts(i, TB)])

            pending = []    # outproj units (tt, n) awaiting emission
            osb_map = {}
            unit_ctr = [0]
            TAIL_TAGS = ["ops", "qkv0", "qkv1", "qkv2"]

            def emit_outproj_unit(tail=False, dve_only=False):
                tt, n = pending.pop(0)
                if tt not in osb_map:
                    osb_map[tt] = opool.tile([128, HID], F16, name="o_sb")
                o_sb = osb_map[tt]
                unit_ctr[0] += 1
                if tail:
                    tag = TAIL_TAGS[unit_ctr[0] % 4]
                else:
                    tag = ["ops", "scr"][unit_ctr[0] % 2]
                if tag == "ops":
                    ps_o = ops_pool.tile([128, TB], F32, name="ps_o", tag="ops")
                elif tag == "scr":
                    ps_o = scrps.tile([128, 4, 128], F32, name="scr", tag="scr")
                else:
                    ps_o = qkvps.tile([128, TB], F32, name=f"ps{tag}", tag=tag)
                nc.tensor.matmul(ps_o[:], ctxT[0][:, ts(tt, 128)],
                                 wo_sb[:, 0, ts(n, 512)], start=True, stop=False)
                nc.tensor.matmul(ps_o[:], ctxT[1][:, ts(tt, 128)],
                                 wo_sb[:, 1, ts(n, 512)], start=False, stop=True)
                if dve_only or unit_ctr[0] % 2 == 0:
                    nc.vector.tensor_copy(o_sb[:, ts(n, 512)], ps_o[:])
                else:
                    nc.scalar.copy(o_sb[:, ts(n, 512)], ps_o[:])
                if n == 3:
                    nc.sync.dma_start(out_d[ts(tt, 128), :], o_sb[:])
                    del osb_map[tt]

            def emit_attn_block(i):
                """Attention for block i, weaving in QKV matmuls of block
                i+1 and outproj units of block i-1 as PE filler."""
                ngrp = len(GROUPS) if i < NTB - 1 else 0
                grp_q = list(range(ngrp))
                if i < NTB - 1:
                    for gi in range(ngrp):
                        emit_h_dma(i + 1, gi)
                    nc.sync.dma_start(cs_sb[:, :, ts(i + 1, TB)],
                                      cs_d[:, :, ts(i + 1, TB)])
                if i == 0:
                    nc.sync.dma_start(mask_sb[:], mask_d)
                    nc.sync.dma_start(ones_sb[:], ones_d)
                    nc.sync.dma_start(wo_sb[:], wo_d)
                if i == 0:
                    # block 0's v-nat runs here, out of its DMA-bound QKV
                    # stream, covering RoPE(0)'s latency tail
                    emit_vnat(0)
                npair = 2 * (i + 1)
                steps_total = npair * QH
                step = [0]

                def filler():
                    steps_left = steps_total - step[0]
                    step[0] += 1
                    if grp_q:
                        for _ in range(2 if i <= 1 else 1):
                            if grp_q:
                                emit_qkv_group(i + 1, grp_q.pop(0))
                        if not grp_q:
                            emit_vnat(i + 1)
                            emit_rope(i + 1)
                        return
                    if pending and steps_left > 0:
                        n_fill = (len(pending) + steps_left - 1) // steps_left
                        for _ in range(min(n_fill, len(pending))):
                            emit_outproj_unit(tail=(i == NTB - 1),
                                              dve_only=(i == NTB - 1))

                for qh in range(QH):
                    ctx_ps = cps_pool.tile([128, TB], F32, name="ctx_ps")
                    den_ps = dps_pool.tile([128, TB], F32, name="den_ps")
                    nkt = 4 * (i + 1)
                    e_tiles = [None] * npair

                    def emit_score_kt(kt, i=i, qh=qh, e_tiles=e_tiles):
                        pj, si = kt // 2, kt % 2
                        if si == 0:
                            e_tiles[pj] = epool.tile([128, 2, TB], F8,
                                                     name="e8", tag="e8")
                        e8t = e_tiles[pj]
                        if i == NTB - 1:
                            stag = ["s_ps", "qkv0", "qkv1", "qkv2"][kt % 4]
                        else:
                            stag = "s_ps"
                        if stag == "s_ps":
                            s_ps = sps_pool.tile([128, TB], F32, name="s_ps")
                        else:
                            s_ps = qkvps.tile([128, TB], F32, name=stag,
                                              tag=stag)
                        j = kt - 4 * i
                        lo = 128 * j if j > 0 else 0
                        nc.tensor.matmul(s_ps[:, lo:TB], k8[:, :, ts(kt, 128)],
                                         q8[qh][:, :, i * TB + lo:
                                                (i + 1) * TB],
                                         start=True, stop=True, perf_mode=DR)
                        nc.scalar.activation(e8t[:, si, lo:TB],
                                             s_ps[:, lo:TB], AF.Exp,
                                             scale=SCALE)
                        if j >= 0:
                            if lo > 0:
                                # the skipped prefix holds stale pool bytes
                                # (can be fp8 NaN/Inf -- x*0 would keep NaN):
                                # zero it explicitly on the idle Pool engine
                                nc.gpsimd.memset(e8t[:, si, 0:lo], 0)
                            nc.vector.tensor_tensor(
                                out=e8t[:, si, lo:lo + 128],
                                in0=e8t[:, si, lo:lo + 128],
                                in1=mask_sb[:, j, lo:lo + 128], op=OP.mult)

                    def emit_consume_piece(ck, npair=npair, ctx_ps=ctx_ps,
                                           den_ps=den_ps, e_tiles=e_tiles):
                        pj = ck // 2
                        e8t = e_tiles[pj]
                        if ck % 2 == 0:
                            nc.tensor.matmul(ctx_ps[:], vhi[:, ts(pj, 2), :],
                                             e8t[:], start=(pj == 0),
                                             stop=False, perf_mode=DR)
                        else:
                            nc.tensor.matmul(ctx_ps[:], vlo[:, ts(pj, 2), :],
                                             e8t[:], start=False,
                                             stop=(pj == npair - 1),
                                             perf_mode=DR)
                            nc.tensor.matmul(den_ps[:], ones_sb[:], e8t[:],
                                             start=(pj == 0),
                                             stop=(pj == npair - 1),
                                             perf_mode=DR)

                    LAG = 4 if npair > 2 else 3
                    for k in range(nkt + LAG):
                        if k < nkt:
                            emit_score_kt(k)
                        ck = k - LAG
                        if ck >= 0:
                            emit_consume_piece(ck)
                            if ck % 2 == 1:
                                filler()

                    recip = t2pool.tile([128, TB], F32, tag="recip",
                                        name="recip")
                    nc.vector.reciprocal(recip[:], den_ps[:])
                    nc.vector.tensor_tensor(out=ctxT[qh][:, ts(i, TB)],
                                            in0=ctx_ps[:], in1=recip[:],
                                            op=OP.mult)

            # ---------------- main pipeline ----------------
            emit_h_dma(0, 0)
            nc.sync.dma_start(w_sb[:, 0:1, :], w_d[:, 0:1, :])
            emit_h_dma(0, 1)
            nc.sync.dma_start(w_sb[:, 1:4, :], w_d[:, 1:4, :])
            nc.sync.dma_start(cs_sb[:, :, ts(0, TB)], cs_d[:, :, ts(0, TB)])
            emit_h_dma(0, 2)
            nc.sync.dma_start(w_sb[:, 4:8, :], w_d[:, 4:8, :])
            emit_h_dma(0, 3)
            nc.sync.dma_start(w_sb[:, 8:12, :], w_d[:, 8:12, :])
            emit_h_dma(0, 4)
            nc.sync.dma_start(w_sb[:, 12:16, :], w_d[:, 12:16, :])
            for gi in range(len(GROUPS0)):
                emit_qkv_group(0, gi)
            emit_rope(0)
            for i in range(NTB):
                emit_attn_block(i)
                pending.extend((tt, n) for tt in range(4 * i, 4 * i + 4)
                               for n in range(4))
            while pending:
                emit_outproj_unit(tail=True)

    nc.compile()
    return nc


_NC_CACHE = None


def _get_nc():
    global _NC_CACHE
    if _NC_CACHE is None:
        _NC_CACHE = _build()
    return _NC_CACHE


def _host_tables(position_ids: np.ndarray):
    pos = np.asarray(position_ids, np.float32)
    inv_freq = (1.0 / (THETA ** (np.arange(0, D, 2, dtype=np.float32) / D)))
    ang = pos[:, None] * inv_freq[None, :]          # [T, 64] f32
    cos = np.cos(ang).T                             # [64, T]
    sin = np.sin(ang).T
    cosT = np.concatenate([cos, cos], axis=0).astype(np.float16)
    sinT = np.concatenate([-sin, sin], axis=0).astype(np.float16)
    return cosT, sinT


def _host_masks():
    # mask for diagonal tile j (keys 128j..128j+128 of the block): columns
    # [0, 128(j+1)): zero where q < k, i.e. col < 128j + row
    r = np.arange(128)[:, None]
    c = np.arange(TB)[None, :]
    m = np.stack([(c - r - 128 * j >= 0) for j in range(4)], axis=1)
    return m.astype(np.float16)                     # [128, 4, TB]


def kernel(hidden_states, position_ids, Wqkv, Wo):
    hidden_states = np.asarray(hidden_states, np.float32)
    Wqkv = np.asarray(Wqkv, np.float32)
    Wo = np.asarray(Wo, np.float32)

    nc = _get_nc()

    hT16 = np.ascontiguousarray(hidden_states.T).astype(np.float16)
    cosT, sinT = _host_tables(position_ids)
    cs16 = np.ascontiguousarray(np.stack([cosT, sinT], axis=1))  # [128,2,T]
    masks = _host_masks()
    ones8 = np.ones((128, 2, 128), dtype=F8NP)

    wq = Wqkv[:, : H * D]
    wk = Wqkv[:, H * D: (H + KV) * D]
    wv = Wqkv[:, (H + KV) * D:]

    in_maps = []
    for c in range(N_CORES):
        kvh = (c * QH) // (H // KV)
        w_cols = np.concatenate(
            [wq[:, (c * QH) * D: (c * QH + 1) * D],
             wq[:, (c * QH + 1) * D: (c * QH + 2) * D],
             wk[:, kvh * D: (kvh + 1) * D],
             wv[:, kvh * D: (kvh + 1) * D]], axis=1)         # [HID, 512]
        w16 = np.ascontiguousarray(
            w_cols.reshape(HCN, 128, 4 * 128).transpose(1, 0, 2)
        ).astype(np.float16)                                 # [128, HCN, 512]
        wo_local = Wo[c * QH * D: (c + 1) * QH * D, :]       # [256, HID]
        wo16 = np.ascontiguousarray(
            wo_local.reshape(2, 128, HID).transpose(1, 0, 2)
        ).astype(np.float16)                                 # [128, 2, HID]
        in_maps.append({
            "hT16": hT16, "w16": w16, "cs16": cs16,
            "mask16": masks, "ones8": ones8, "wo16": wo16,
        })

    res = bass_utils.run_bass_kernel_spmd(nc, in_maps,
                                          core_ids=list(range(N_CORES)))
    parts = np.stack([res.results[c]["out16"].astype(np.float32)
                      for c in range(N_CORES)], 0)
    return parts.sum(axis=0, dtype=np.float32)


# revision 45
# speedup vs baseline: 1.3494x; 1.0096x over previous
"""Bass/Trainium2 kernel for BailingAttention (GQA prefill, causal, RoPE).

Sharding: tensor-parallel over heads across 8 NeuronCores. Each core computes
2 query heads + its group's shared KV head end-to-end (QKV projection, RoPE,
causal attention, output projection) and writes a partial [T, HID] fp16
output; the host sums the 8 partials (the row-parallel all-reduce).

Precision plan (gate is rel-err < 2e-2; this lands ~1.4e-2):
  - QKV + output projections: fp16 x fp16 matmuls (1 PE cycle/row).
  - Scores: fp8 DoubleRow, one instruction per key tile computing
    k8^T(q_hi + q_lo) -- the two DoubleRow subtile slots carry a hi/lo fp8
    split of q, so the q side is ~exact and only k carries fp8 noise.
    0.5 cycles/row: 2x fp32r.
  - exp: ACT engine writes fp8e4 directly; softmax numerator/denominator use
    the SAME quantized e so weight-quantization largely cancels.
  - PV: two DoubleRow instructions per key-tile PAIR: (v_hi[2j],v_hi[2j+1])
    and (v_lo[...]) against the e pair -- v is hi/lo-split (~exact), e noise
    cancels through the denominator. 2x fp32r.
  - Denominator: DoubleRow over e pairs with a ones stationary: 4x fp32r.

Schedule: one fused software pipeline. Block 0's QKV runs first; thereafter
attention for block i runs with the QKV matmuls of block i+1 and the
output-projection units of block i-1 woven between its score/PV steps as
tensor-engine filler, so the PE stays dense (and at full p-state clock)
while ACT works through the exps. All of block i+1's hidden-stream DMAs are
prefetched at the start of attention i; RoPE for block i+1 is emitted as
soon as its last hc lands so its latency hides under block i's remaining
pairs. Output-projection units rotate over four PSUM banks (the scratch
bank plus the three QKV banks, which are idle between accumulations).

Layouts on device (partition dim first):
  hT16    [HID, T] fp16 (host-transposed)  -> moving operand of QKV matmuls
  q8      [D, 2(hi,lo), T] fp8 per head    -> scores moving
  k8      [D, 2(dup), T] fp8               -> scores stationary slices
  v hi/lo [T-part, kt, D] fp8 natural      -> PV stationary; produced by
          per-token-tile matmuls (stationary = hT slice) -- no PE transpose
  e8      [kt, 2, TB] fp8 pair tiles       -> PV/denominator moving
  ctxT    [D, T] fp16                      -> output projection stationary
"""

import numpy as np
import ml_dtypes

import concourse.bass as bass
import concourse.mybir as mybir
import concourse.tile as tile
from concourse import bacc, bass_utils
from concourse.bass import ts

F32 = mybir.dt.float32
F16 = mybir.dt.float16
F8 = mybir.dt.float8e4
AF = mybir.ActivationFunctionType
OP = mybir.AluOpType
DR = mybir.MatmulPerfMode.DoubleRow

H, KV, D, HID, T = 16, 4, 128, 2048, 2048
THETA = 10000.0
N_CORES = 8
QH = H // N_CORES            # query heads per core = 2
TB = 512                     # token block
NTB = T // TB                # 4
HCN = HID // 128             # 16 h-chunks
SCALE = float(D) ** -0.5
F8NP = ml_dtypes.float8_e4m3

# hc group layout: block 0 ramps up (small first DMA so the first matmul
# starts early); other blocks use 4-chunk groups
GROUPS0 = [(0, 1), (1, 3), (4, 4), (8, 4), (12, 4)]
GROUPS = [(0, 4), (4, 4), (8, 4), (12, 4)]


def _build():
    nc = bacc.Bacc("TRN2", target_bir_lowering=False, debug=False,
                   num_devices=N_CORES)

    hT_d = nc.dram_tensor("hT16", [HID, T], F16, kind="ExternalInput").ap()
    w_d = nc.dram_tensor("w16", [128, HCN, 4 * 128], F16, kind="ExternalInput").ap()
    cs_d = nc.dram_tensor("cs16", [128, 2, T], F16, kind="ExternalInput").ap()
    mask_d = nc.dram_tensor("mask16", [128, 4, TB], F16, kind="ExternalInput").ap()
    ones_d = nc.dram_tensor("ones8", [128, 2, 128], F8, kind="ExternalInput").ap()
    wo_d = nc.dram_tensor("wo16", [128, 2, HID], F16, kind="ExternalInput").ap()
    out_d = nc.dram_tensor("out16", [T, HID], F16, kind="ExternalOutput").ap()

    hT_view = hT_d.rearrange("(hc p) t -> hc p t", p=128)

    with tile.TileContext(nc) as tc:
        with tc.tile_pool(name="const", bufs=1) as cpool, \
             tc.tile_pool(name="acts", bufs=1) as apool, \
             tc.tile_pool(name="hstream", bufs=8) as hpool, \
             tc.tile_pool(name="rtmp", bufs=3) as tpool, \
             tc.tile_pool(name="p2e", bufs=10) as epool, \
             tc.tile_pool(name="p2tmp", bufs=2) as t2pool, \
             tc.tile_pool(name="p3out", bufs=3) as opool, \
             tc.tile_pool(name="qkvps", bufs=1, space="PSUM") as qkvps, \
             tc.tile_pool(name="scrps", bufs=1, space="PSUM") as scrps, \
             tc.tile_pool(name="sps", bufs=1, space="PSUM") as sps_pool, \
             tc.tile_pool(name="ops", bufs=1, space="PSUM") as ops_pool, \
             tc.tile_pool(name="cps", bufs=1, space="PSUM") as cps_pool, \
             tc.tile_pool(name="dps", bufs=1, space="PSUM") as dps_pool:

            w_sb = cpool.tile([128, HCN, 4 * 128], F16)
            cs_sb = cpool.tile([128, 2, T], F16)
            mask_sb = cpool.tile([128, 4, TB], F16)
            ones_sb = cpool.tile([128, 2, 128], F8)
            wo_sb = cpool.tile([128, 2, HID], F16)

            q8 = [apool.tile([128, 2, T], F8, name=f"q8_{i}", tag=f"q8_{i}")
                  for i in range(QH)]
            k8 = apool.tile([128, 2, T], F8)
            vhi = apool.tile([128, 16, 128], F8)
            vlo = apool.tile([128, 16, 128], F8)
            ctxT = [apool.tile([128, T], F16, name=f"ctxT{i}", tag=f"ctxT{i}")
                    for i in range(QH)]

            # ---------------- emit helpers ----------------
            qkv_live = {}   # block -> [ps_q0, ps_q1, ps_k]
            vnat_live = {}  # block -> ps_v (from scratch pool)
            h_tiles = {}    # (block, group-idx) -> sbuf tile
            h_keep = {}     # retained h tiles for deferred v-nat

            def emit_h_dma(i, gi):
                hc0, wid = (GROUPS0 if i == 0 else GROUPS)[gi]
                hT_t = hpool.tile([128, wid, TB], F16, tag=f"h{wid}")
                src = hT_view[hc0:hc0 + wid, :, ts(i, TB)]
                nc.sync.dma_start(hT_t[:], src.rearrange("g p t -> p g t"))
                h_tiles[(i, gi)] = hT_t

            def emit_qkv_group(i, gi):
                hc0, wid = (GROUPS0 if i == 0 else GROUPS)[gi]
                if gi == 0:
                    qkv_live[i] = [qkvps.tile([128, TB], F32, name=f"psqkv{n}",
                                              tag=f"qkv{n}") for n in range(3)]
                ps_qkv = qkv_live[i]
                hT_t = h_tiles.pop((i, gi))
                h_keep[(i, gi)] = hT_t
                for j in range(wid):
                    hc = hc0 + j
                    for n in range(3):
                        nc.tensor.matmul(ps_qkv[n][:], w_sb[:, hc, ts(n, 128)],
                                         hT_t[:, j, :], start=(hc == 0),
                                         stop=(hc == HCN - 1))

            def emit_vnat(i):
                # v in natural layout from the retained h tiles. tt-OUTER:
                # each PSUM sub-region's accumulation group must run
                # start->stop consecutively (interleaved same-bank
                # accumulation groups produce garbage on hardware).
                vnat_live[i] = scrps.tile([128, 4, 128], F32, name="scr",
                                          tag="scr")
                ps_v = vnat_live[i]
                groups = GROUPS0 if i == 0 else GROUPS
                for tt in range(4):
                    for gi, (hc0, wid) in enumerate(groups):
                        hT_t = h_keep[(i, gi)]
                        for j in range(wid):
                            hc = hc0 + j
                            nc.tensor.matmul(ps_v[:, tt, :],
                                             hT_t[:, j, ts(tt, 128)],
                                             w_sb[:, hc, ts(3, 128)],
                                             start=(hc == 0),
                                             stop=(hc == HCN - 1))
                for gi in range(len(groups)):
                    h_keep.pop((i, gi))
                # v hi/lo casts
                nc.vector.tensor_copy(vhi[:, ts(i, 4), :], ps_v[:])
                nc.vector.tensor_tensor(out=vlo[:, ts(i, 4), :], in0=ps_v[:],
                                        in1=vhi[:, ts(i, 4), :], op=OP.subtract)
                vnat_live.pop(i)

            def emit_rope(i):
                ps_qkv = qkv_live.pop(i)
                # fp16 RoPE; k chunk (slot 2) first so scores can start
                # after k + q0 land
                x_all = tpool.tile([128, 3, TB], F16, tag="ropex")
                for n in (2, 0, 1):
                    nc.vector.tensor_copy(x_all[:, n, :], ps_qkv[n][:])
                xsw = tpool.tile([128, 3, TB], F16, tag="ropesw")
                nc.scalar.dma_start(xsw[0:64, :, :], x_all[64:128, :, :])
                nc.scalar.dma_start(xsw[64:128, :, :], x_all[0:64, :, :])
                for n in (2, 0, 1):
                    t2 = tpool.tile([128, TB], F16, tag="ropet2")
                    nc.vector.tensor_tensor(out=t2[:], in0=xsw[:, n, :],
                                            in1=cs_sb[:, 1, ts(i, TB)],
                                            op=OP.mult)
                    m1 = tpool.tile([128, TB], F16, tag="ropem1")
                    nc.gpsimd.tensor_tensor(out=m1[:], in0=x_all[:, n, :],
                                            in1=cs_sb[:, 0, ts(i, TB)],
                                            op=OP.mult)
                    if n < QH:
                        qf = tpool.tile([128, TB], F16, tag="ropeqf")
                        nc.vector.tensor_tensor(out=qf[:], in0=m1[:], in1=t2[:],
                                                op=OP.add)
                        nc.gpsimd.tensor_copy(q8[n][:, 0, ts(i, TB)], qf[:])
                        nc.vector.tensor_tensor(
                            out=q8[n][:, 1, ts(i, TB)], in0=qf[:],
                            in1=q8[n][:, 0, ts(i, TB)], op=OP.subtract)
                    else:
                        nc.vector.tensor_tensor(out=k8[:, 0, ts(i, TB)],
                                                in0=m1[:], in1=t2[:], op=OP.add)
                        nc.scalar.copy(k8[:, 1, ts(i, TB)], k8[:, 0, ts(i, TB)])

            pending = []    # outproj units (tt, n) awaiting emission
            osb_map = {}
            unit_ctr = [0]
            TAIL_TAGS = ["ops", "qkv0", "qkv1", "qkv2"]

            def emit_outproj_unit(tail=False, dve_only=False):
                tt, n = pending.pop(0)
                if tt not in osb_map:
                    osb_map[tt] = opool.tile([128, HID], F16, name="o_sb")
                o_sb = osb_map[tt]
                unit_ctr[0] += 1
                if tail:
                    tag = TAIL_TAGS[unit_ctr[0] % 4]
                else:
                    tag = ["ops", "scr"][unit_ctr[0] % 2]
                if tag == "ops":
                    ps_o = ops_pool.tile([128, TB], F32, name="ps_o", tag="ops")
                elif tag == "scr":
                    ps_o = scrps.tile([128, 4, 128], F32, name="scr", tag="scr")
                else:
                    ps_o = qkvps.tile([128, TB], F32, name=f"ps{tag}", tag=tag)
                nc.tensor.matmul(ps_o[:], ctxT[0][:, ts(tt, 128)],
                                 wo_sb[:, 0, ts(n, 512)], start=True, stop=False)
                nc.tensor.matmul(ps_o[:], ctxT[1][:, ts(tt, 128)],
                                 wo_sb[:, 1, ts(n, 512)], start=False, stop=True)
                if dve_only or unit_ctr[0] % 2 == 0:
                    nc.vector.tensor_copy(o_sb[:, ts(n, 512)], ps_o[:])
                else:
                    nc.scalar.copy(o_sb[:, ts(n, 512)], ps_o[:])
                if n == 3:
                    nc.sync.dma_start(out_d[ts(tt, 128), :], o_sb[:])
                    del osb_map[tt]

            def emit_attn_block(i):
                """Attention for block i, weaving in QKV matmuls of block
                i+1 and outproj units of block i-1 as PE filler."""
                ngrp = len(GROUPS) if i < NTB - 1 else 0
                grp_q = list(range(ngrp))
                if i < NTB - 1:
                    for gi in range(ngrp):
                        emit_h_dma(i + 1, gi)
                    nc.sync.dma_start(cs_sb[:, :, ts(i + 1, TB)],
                                      cs_d[:, :, ts(i + 1, TB)])
                if i == 0:
                    nc.sync.dma_start(mask_sb[:], mask_d)
                    nc.sync.dma_start(ones_sb[:], ones_d)
                    nc.sync.dma_start(wo_sb[:], wo_d)
                if i == 0:
                    # block 0's v-nat runs here, out of its DMA-bound QKV
                    # stream, covering RoPE(0)'s latency tail
                    emit_vnat(0)
                npair = 2 * (i + 1)
                steps_total = npair * QH
                step = [0]

                def filler():
                    steps_left = steps_total - step[0]
                    step[0] += 1
                    if grp_q:
                        for _ in range(2 if i <= 1 else 1):
                            if grp_q:
                                emit_qkv_group(i + 1, grp_q.pop(0))
                        if not grp_q:
                            emit_vnat(i + 1)
                            emit_rope(i + 1)
                        return
                    if pending and steps_left > 0:
                        n_fill = (len(pending) + steps_left - 1) // steps_left
                        for _ in range(min(n_fill, len(pending))):
                            emit_outproj_unit(tail=(i == NTB - 1),
                                              dve_only=(i == NTB - 1))

                for qh in range(QH):
                    ctx_ps = cps_pool.tile([128, TB], F32, name="ctx_ps")
                    den_ps = dps_pool.tile([128, TB], F32, name="den_ps")
                    nkt = 4 * (i + 1)
                    e_tiles = [None] * npair

                    def emit_score_kt(kt, i=i, qh=qh, e_tiles=e_tiles):
                        pj, si = kt // 2, kt % 2
                        if si == 0:
                            e_tiles[pj] = epool.tile([128, 2, TB], F8,
                                                     name="e8", tag="e8")
                        e8t = e_tiles[pj]
                        if i == NTB - 1:
                            stag = ["s_ps", "qkv0", "qkv1", "qkv2"][kt % 4]
                        else:
                            stag = "s_ps"
                        if stag == "s_ps":
                            s_ps = sps_pool.tile([128, TB], F32, name="s_ps")
                        else:
                            s_ps = qkvps.tile([128, TB], F32, name=stag,
                                              tag=stag)
                        j = kt - 4 * i
                        lo = 128 * j if j > 0 else 0
                        nc.tensor.matmul(s_ps[:, lo:TB], k8[:, :, ts(kt, 128)],
                                         q8[qh][:, :, i * TB + lo:
                                                (i + 1) * TB],
                                         start=True, stop=True, perf_mode=DR)
                        nc.scalar.activation(e8t[:, si, lo:TB],
                                             s_ps[:, lo:TB], AF.Exp,
                                             scale=SCALE)
                        if j >= 0:
                            if lo > 0:
                                # the skipped prefix holds stale pool bytes
                                # (can be fp8 NaN/Inf -- x*0 would keep NaN):
                                # zero it explicitly on the idle Pool engine
                                nc.gpsimd.memset(e8t[:, si, 0:lo], 0)
                            nc.vector.tensor_tensor(
                                out=e8t[:, si, lo:lo + 128],
                                in0=e8t[:, si, lo:lo + 128],
                                in1=mask_sb[:, j, lo:lo + 128], op=OP.mult)

                    def emit_consume_piece(ck, npair=npair, ctx_ps=ctx_ps,
                                           den_ps=den_ps, e_tiles=e_tiles):
                        pj = ck // 2
                        e8t = e_tiles[pj]
                        if ck % 2 == 0:
                            nc.tensor.matmul(ctx_ps[:], vhi[:, ts(pj, 2), :],
                                             e8t[:], start=(pj == 0),
                                             stop=False, perf_mode=DR)
                        else:
                            nc.tensor.matmul(ctx_ps[:], vlo[:, ts(pj, 2), :],
                                             e8t[:], start=False,
                                             stop=(pj == npair - 1),
                                             perf_mode=DR)
                            nc.tensor.matmul(den_ps[:], ones_sb[:], e8t[:],
                                             start=(pj == 0),
                                             stop=(pj == npair - 1),
                                             perf_mode=DR)

                    LAG = 4 if npair > 2 else 3
                    for k in range(nkt + LAG):
                        if k < nkt:
                            emit_score_kt(k)
                        ck = k - LAG
                        if ck >= 0:
                            emit_consume_piece(ck)
                            if ck % 2 == 1:
                                filler()

                    recip = t2pool.tile([128, TB], F32, tag="recip",
                                        name="recip")
                    nc.vector.reciprocal(recip[:], den_ps[:])
                    nc.vector.tensor_tensor(out=ctxT[qh][:, ts(i, TB)],
                                            in0=ctx_ps[:], in1=recip[:],
                                            op=OP.mult)

            # ---------------- main pipeline ----------------
            emit_h_dma(0, 0)
            nc.sync.dma_start(w_sb[:, 0:1, :], w_d[:, 0:1, :])
            emit_h_dma(0, 1)
            nc.sync.dma_start(w_sb[:, 1:4, :], w_d[:, 1:4, :])
            nc.sync.dma_start(cs_sb[:, :, ts(0, TB)], cs_d[:, :, ts(0, TB)])
            emit_h_dma(0, 2)
            nc.sync.dma_start(w_sb[:, 4:8, :], w_d[:, 4:8, :])
            emit_h_dma(0, 3)
            nc.sync.dma_start(w_sb[:, 8:12, :], w_d[:, 8:12, :])
            emit_h_dma(0, 4)
            nc.sync.dma_start(w_sb[:, 12:16, :], w_d[:, 12:16, :])
            for gi in range(len(GROUPS0)):
                emit_qkv_group(0, gi)
            emit_rope(0)
            for i in range(NTB):
                emit_attn_block(i)
                pending.extend((tt, n) for tt in range(4 * i, 4 * i + 4)
                               for n in range(4))
            while pending:
                emit_outproj_unit(tail=True)

    nc.compile()
    return nc


_NC_CACHE = None


def _get_nc():
    global _NC_CACHE
    if _NC_CACHE is None:
        _NC_CACHE = _build()
    return _NC_CACHE


def _host_tables(position_ids: np.ndarray):
    pos = np.asarray(position_ids, np.float32)
    inv_freq = (1.0 / (THETA ** (np.arange(0, D, 2, dtype=np.float32) / D)))
    ang = pos[:, None] * inv_freq[None, :]          # [T, 64] f32
    cos = np.cos(ang).T                             # [64, T]
    sin = np.sin(ang).T
    cosT = np.concatenate([cos, cos], axis=0).astype(np.float16)
    sinT = np.concatenate([-sin, sin], axis=0).astype(np.float16)
    return cosT, sinT


def _host_masks():
    # mask for diagonal tile j (keys 128j..128j+128 of the block): columns
    # [0, 128(j+1)): zero where q < k, i.e. col < 128j + row
    r = np.arange(128)[:, None]
    c = np.arange(TB)[None, :]
    m = np.stack([(c - r - 128 * j >= 0) for j in range(4)], axis=1)
    return m.astype(np.float16)                     # [128, 4, TB]


def kernel(hidden_states, position_ids, Wqkv, Wo):
    hidden_states = np.asarray(hidden_states, np.float32)
    Wqkv = np.asarray(Wqkv, np.float32)
    Wo = np.asarray(Wo, np.float32)

    nc = _get_nc()

    hT16 = np.ascontiguousarray(hidden_states.T).astype(np.float16)
    cosT, sinT = _host_tables(position_ids)
    cs16 = np.ascontiguousarray(np.stack([cosT, sinT], axis=1))  # [128,2,T]
    masks = _host_masks()
    ones8 = np.ones((128, 2, 128), dtype=F8NP)

    wq = Wqkv[:, : H * D]
    wk = Wqkv[:, H * D: (H + KV) * D]
    wv = Wqkv[:, (H + KV) * D:]

    in_maps = []
    for c in range(N_CORES):
        kvh = (c * QH) // (H // KV)
        w_cols = np.concatenate(
            [wq[:, (c * QH) * D: (c * QH + 1) * D],
             wq[:, (c * QH + 1) * D: (c * QH + 2) * D],
             wk[:, kvh * D: (kvh + 1) * D],
             wv[:, kvh * D: (kvh + 1) * D]], axis=1)         # [HID, 512]
        w16 = np.ascontiguousarray(
            w_cols.reshape(HCN, 128, 4 * 128).transpose(1, 0, 2)
        ).astype(np.float16)                                 # [128, HCN, 512]
        wo_local = Wo[c * QH * D: (c + 1) * QH * D, :]       # [256, HID]
        wo16 = np.ascontiguousarray(
            wo_local.reshape(2, 128, HID).transpose(1, 0, 2)
        ).astype(np.float16)                                 # [128, 2, HID]
        in_maps.append({
            "hT16": hT16, "w16": w16, "cs16": cs16,
            "mask16": masks, "ones8": ones8, "wo16": wo16,
        })

    res = bass_utils.run_bass_kernel_spmd(nc, in_maps,
                                          core_ids=list(range(N_CORES)))
    parts = np.stack([res.results[c]["out16"].astype(np.float32)
                      for c in range(N_CORES)], 0)
    return parts.sum(axis=0, dtype=np.float32)


# revision 46
# speedup vs baseline: 1.3622x; 1.0095x over previous
"""Bass/Trainium2 kernel for BailingAttention (GQA prefill, causal, RoPE).

Sharding: tensor-parallel over heads across 8 NeuronCores. Each core computes
2 query heads + its group's shared KV head end-to-end (QKV projection, RoPE,
causal attention, output projection) and writes a partial [T, HID] fp16
output; the host sums the 8 partials (the row-parallel all-reduce).

Precision plan (gate is rel-err < 2e-2; this lands ~1.4e-2):
  - QKV + output projections: fp16 x fp16 matmuls (1 PE cycle/row).
  - Scores: fp8 DoubleRow, one instruction per key tile computing
    k8^T(q_hi + q_lo) -- the two DoubleRow subtile slots carry a hi/lo fp8
    split of q, so the q side is ~exact and only k carries fp8 noise.
    0.5 cycles/row: 2x fp32r.
  - exp: ACT engine writes fp8e4 directly; softmax numerator/denominator use
    the SAME quantized e so weight-quantization largely cancels.
  - PV: two DoubleRow instructions per key-tile PAIR: (v_hi[2j],v_hi[2j+1])
    and (v_lo[...]) against the e pair -- v is hi/lo-split (~exact), e noise
    cancels through the denominator. 2x fp32r.
  - Denominator: DoubleRow over e pairs with a ones stationary: 4x fp32r.

Schedule: one fused software pipeline. Block 0's QKV runs first; thereafter
attention for block i runs with the QKV matmuls of block i+1 and the
output-projection units of block i-1 woven between its score/PV steps as
tensor-engine filler, so the PE stays dense (and at full p-state clock)
while ACT works through the exps. All of block i+1's hidden-stream DMAs are
prefetched at the start of attention i; RoPE for block i+1 is emitted as
soon as its last hc lands so its latency hides under block i's remaining
pairs. Output-projection units rotate over four PSUM banks (the scratch
bank plus the three QKV banks, which are idle between accumulations).

Layouts on device (partition dim first):
  hT16    [HID, T] fp16 (host-transposed)  -> moving operand of QKV matmuls
  q8      [D, 2(hi,lo), T] fp8 per head    -> scores moving
  k8      [D, 2(dup), T] fp8               -> scores stationary slices
  v hi/lo [T-part, kt, D] fp8 natural      -> PV stationary; produced by
          per-token-tile matmuls (stationary = hT slice) -- no PE transpose
  e8      [kt, 2, TB] fp8 pair tiles       -> PV/denominator moving
  ctxT    [D, T] fp16                      -> output projection stationary
"""

import numpy as np
import ml_dtypes

import concourse.bass as bass
import concourse.mybir as mybir
import concourse.tile as tile
from concourse import bacc, bass_utils
from concourse.bass import ts

F32 = mybir.dt.float32
F16 = mybir.dt.float16
F8 = mybir.dt.float8e4
AF = mybir.ActivationFunctionType
OP = mybir.AluOpType
DR = mybir.MatmulPerfMode.DoubleRow

H, KV, D, HID, T = 16, 4, 128, 2048, 2048
THETA = 10000.0
N_CORES = 8
QH = H // N_CORES            # query heads per core = 2
TB = 512                     # token block
NTB = T // TB                # 4
HCN = HID // 128             # 16 h-chunks
SCALE = float(D) ** -0.5
F8NP = ml_dtypes.float8_e4m3

# hc group layout: block 0 ramps up (small first DMA so the first matmul
# starts early); other blocks use 4-chunk groups
GROUPS0 = [(0, 1), (1, 3), (4, 4), (8, 4), (12, 4)]
GROUPS = [(0, 4), (4, 4), (8, 4), (12, 4)]


def _build():
    nc = bacc.Bacc("TRN2", target_bir_lowering=False, debug=False,
                   num_devices=N_CORES)

    hT_d = nc.dram_tensor("hT16", [HID, T], F16, kind="ExternalInput").ap()
    w_d = nc.dram_tensor("w16", [128, HCN, 4 * 128], F16, kind="ExternalInput").ap()
    cs_d = nc.dram_tensor("cs16", [128, 2, T], F16, kind="ExternalInput").ap()
    mask_d = nc.dram_tensor("mask16", [128, 4, TB], F16, kind="ExternalInput").ap()
    ones_d = nc.dram_tensor("ones8", [128, 2, 128], F8, kind="ExternalInput").ap()
    wo_d = nc.dram_tensor("wo16", [128, 2, HID], F16, kind="ExternalInput").ap()
    out_d = nc.dram_tensor("out16", [T, HID], F16, kind="ExternalOutput").ap()

    hT_view = hT_d.rearrange("(hc p) t -> hc p t", p=128)

    with tile.TileContext(nc) as tc:
        with tc.tile_pool(name="const", bufs=1) as cpool, \
             tc.tile_pool(name="acts", bufs=1) as apool, \
             tc.tile_pool(name="hstream", bufs=8) as hpool, \
             tc.tile_pool(name="rtmp", bufs=3) as tpool, \
             tc.tile_pool(name="p2e", bufs=10) as epool, \
             tc.tile_pool(name="p2tmp", bufs=2) as t2pool, \
             tc.tile_pool(name="p3out", bufs=3) as opool, \
             tc.tile_pool(name="qkvps", bufs=1, space="PSUM") as qkvps, \
             tc.tile_pool(name="scrps", bufs=1, space="PSUM") as scrps, \
             tc.tile_pool(name="sps", bufs=1, space="PSUM") as sps_pool, \
             tc.tile_pool(name="ops", bufs=1, space="PSUM") as ops_pool, \
             tc.tile_pool(name="cps", bufs=1, space="PSUM") as cps_pool, \
             tc.tile_pool(name="dps", bufs=1, space="PSUM") as dps_pool:

            w_sb = cpool.tile([128, HCN, 4 * 128], F16)
            cs_sb = cpool.tile([128, 2, T], F16)
            mask_sb = cpool.tile([128, 4, TB], F16)
            ones_sb = cpool.tile([128, 2, 128], F8)
            wo_sb = cpool.tile([128, 2, HID], F16)

            q8 = [apool.tile([128, 2, T], F8, name=f"q8_{i}", tag=f"q8_{i}")
                  for i in range(QH)]
            k8 = apool.tile([128, 2, T], F8)
            vhi = apool.tile([128, 16, 128], F8)
            vlo = apool.tile([128, 16, 128], F8)
            ctxT = [apool.tile([128, T], F16, name=f"ctxT{i}", tag=f"ctxT{i}")
                    for i in range(QH)]

            # ---------------- emit helpers ----------------
            qkv_live = {}   # block -> [ps_q0, ps_q1, ps_k]
            vnat_live = {}  # block -> ps_v (from scratch pool)
            h_tiles = {}    # (block, group-idx) -> sbuf tile
            h_keep = {}     # retained h tiles for deferred v-nat

            def emit_h_dma(i, gi):
                hc0, wid = (GROUPS0 if i == 0 else GROUPS)[gi]
                hT_t = hpool.tile([128, wid, TB], F16, tag=f"h{wid}")
                src = hT_view[hc0:hc0 + wid, :, ts(i, TB)]
                nc.sync.dma_start(hT_t[:], src.rearrange("g p t -> p g t"))
                h_tiles[(i, gi)] = hT_t

            def emit_qkv_group(i, gi):
                hc0, wid = (GROUPS0 if i == 0 else GROUPS)[gi]
                if gi == 0:
                    qkv_live[i] = [qkvps.tile([128, TB], F32, name=f"psqkv{n}",
                                              tag=f"qkv{n}") for n in range(3)]
                ps_qkv = qkv_live[i]
                hT_t = h_tiles.pop((i, gi))
                h_keep[(i, gi)] = hT_t
                for j in range(wid):
                    hc = hc0 + j
                    for n in range(3):
                        nc.tensor.matmul(ps_qkv[n][:], w_sb[:, hc, ts(n, 128)],
                                         hT_t[:, j, :], start=(hc == 0),
                                         stop=(hc == HCN - 1))

            def emit_vnat(i, tts=(0, 1, 2, 3)):
                # v in natural layout from the retained h tiles. tt-OUTER:
                # each PSUM sub-region's accumulation group must run
                # start->stop consecutively (interleaved same-bank
                # accumulation groups produce garbage on hardware).
                if i not in vnat_live:
                    vnat_live[i] = scrps.tile([128, 4, 128], F32, name="scr",
                                              tag="scr")
                ps_v = vnat_live[i]
                groups = GROUPS0 if i == 0 else GROUPS
                for tt in tts:
                    for gi, (hc0, wid) in enumerate(groups):
                        hT_t = h_keep[(i, gi)]
                        for j in range(wid):
                            hc = hc0 + j
                            nc.tensor.matmul(ps_v[:, tt, :],
                                             hT_t[:, j, ts(tt, 128)],
                                             w_sb[:, hc, ts(3, 128)],
                                             start=(hc == 0),
                                             stop=(hc == HCN - 1))
                if tts[-1] == 3:
                    for gi in range(len(groups)):
                        h_keep.pop((i, gi))
                    # v hi/lo casts
                    nc.vector.tensor_copy(vhi[:, ts(i, 4), :], ps_v[:])
                    nc.vector.tensor_tensor(out=vlo[:, ts(i, 4), :],
                                            in0=ps_v[:],
                                            in1=vhi[:, ts(i, 4), :],
                                            op=OP.subtract)
                    vnat_live.pop(i)

            def emit_rope(i):
                ps_qkv = qkv_live.pop(i)
                # fp16 RoPE; k chunk (slot 2) first so scores can start
                # after k + q0 land
                x_all = tpool.tile([128, 3, TB], F16, tag="ropex")
                for n in (2, 0, 1):
                    nc.vector.tensor_copy(x_all[:, n, :], ps_qkv[n][:])
                xsw = tpool.tile([128, 3, TB], F16, tag="ropesw")
                nc.scalar.dma_start(xsw[0:64, :, :], x_all[64:128, :, :])
                nc.scalar.dma_start(xsw[64:128, :, :], x_all[0:64, :, :])
                for n in (2, 0, 1):
                    t2 = tpool.tile([128, TB], F16, tag="ropet2")
                    nc.vector.tensor_tensor(out=t2[:], in0=xsw[:, n, :],
                                            in1=cs_sb[:, 1, ts(i, TB)],
                                            op=OP.mult)
                    m1 = tpool.tile([128, TB], F16, tag="ropem1")
                    nc.gpsimd.tensor_tensor(out=m1[:], in0=x_all[:, n, :],
                                            in1=cs_sb[:, 0, ts(i, TB)],
                                            op=OP.mult)
                    if n < QH:
                        qf = tpool.tile([128, TB], F16, tag="ropeqf")
                        nc.vector.tensor_tensor(out=qf[:], in0=m1[:], in1=t2[:],
                                                op=OP.add)
                        nc.gpsimd.tensor_copy(q8[n][:, 0, ts(i, TB)], qf[:])
                        nc.vector.tensor_tensor(
                            out=q8[n][:, 1, ts(i, TB)], in0=qf[:],
                            in1=q8[n][:, 0, ts(i, TB)], op=OP.subtract)
                    else:
                        nc.vector.tensor_tensor(out=k8[:, 0, ts(i, TB)],
                                                in0=m1[:], in1=t2[:], op=OP.add)
                        nc.scalar.copy(k8[:, 1, ts(i, TB)], k8[:, 0, ts(i, TB)])

            vnat_q = []     # deferred v-nat half-emissions
            pending = []    # outproj units (tt, n) awaiting emission
            osb_map = {}
            unit_ctr = [0]
            TAIL_TAGS = ["ops", "qkv0", "qkv1", "qkv2"]

            def emit_outproj_unit(tail=False, dve_only=False):
                tt, n = pending.pop(0)
                if tt not in osb_map:
                    osb_map[tt] = opool.tile([128, HID], F16, name="o_sb")
                o_sb = osb_map[tt]
                unit_ctr[0] += 1
                if tail:
                    tag = TAIL_TAGS[unit_ctr[0] % 4]
                else:
                    tag = ["ops", "scr"][unit_ctr[0] % 2]
                if tag == "ops":
                    ps_o = ops_pool.tile([128, TB], F32, name="ps_o", tag="ops")
                elif tag == "scr":
                    ps_o = scrps.tile([128, 4, 128], F32, name="scr", tag="scr")
                else:
                    ps_o = qkvps.tile([128, TB], F32, name=f"ps{tag}", tag=tag)
                nc.tensor.matmul(ps_o[:], ctxT[0][:, ts(tt, 128)],
                                 wo_sb[:, 0, ts(n, 512)], start=True, stop=False)
                nc.tensor.matmul(ps_o[:], ctxT[1][:, ts(tt, 128)],
                                 wo_sb[:, 1, ts(n, 512)], start=False, stop=True)
                if dve_only or unit_ctr[0] % 2 == 0:
                    nc.vector.tensor_copy(o_sb[:, ts(n, 512)], ps_o[:])
                else:
                    nc.scalar.copy(o_sb[:, ts(n, 512)], ps_o[:])
                if n == 3:
                    nc.sync.dma_start(out_d[ts(tt, 128), :], o_sb[:])
                    del osb_map[tt]

            def emit_attn_block(i):
                """Attention for block i, weaving in QKV matmuls of block
                i+1 and outproj units of block i-1 as PE filler."""
                ngrp = len(GROUPS) if i < NTB - 1 else 0
                grp_q = list(range(ngrp))
                if i < NTB - 1:
                    for gi in range(ngrp):
                        emit_h_dma(i + 1, gi)
                    nc.sync.dma_start(cs_sb[:, :, ts(i + 1, TB)],
                                      cs_d[:, :, ts(i + 1, TB)])
                if i == 0:
                    nc.sync.dma_start(mask_sb[:], mask_d)
                    nc.sync.dma_start(ones_sb[:], ones_d)
                    nc.sync.dma_start(wo_sb[:], wo_d)
                if i == 0:
                    # block 0's v-nat runs here, out of its DMA-bound QKV
                    # stream, covering RoPE(0)'s latency tail
                    emit_vnat(0)
                npair = 2 * (i + 1)
                steps_total = npair * QH
                step = [0]

                def filler():
                    steps_left = steps_total - step[0]
                    step[0] += 1
                    if grp_q:
                        for _ in range(2 if i <= 1 else 1):
                            if grp_q:
                                emit_qkv_group(i + 1, grp_q.pop(0))
                        if not grp_q:
                            emit_rope(i + 1)
                            vnat_q.extend([(i + 1, (0, 1)), (i + 1, (2, 3))])
                        return
                    if vnat_q:
                        blk, tts = vnat_q.pop(0)
                        emit_vnat(blk, tts)
                        return
                    if pending and steps_left > 0:
                        n_fill = (len(pending) + steps_left - 1) // steps_left
                        for _ in range(min(n_fill, len(pending))):
                            emit_outproj_unit(tail=(i == NTB - 1),
                                              dve_only=(i == NTB - 1))

                for qh in range(QH):
                    ctx_ps = cps_pool.tile([128, TB], F32, name="ctx_ps")
                    den_ps = dps_pool.tile([128, TB], F32, name="den_ps")
                    nkt = 4 * (i + 1)
                    e_tiles = [None] * npair

                    def emit_score_kt(kt, i=i, qh=qh, e_tiles=e_tiles):
                        pj, si = kt // 2, kt % 2
                        if si == 0:
                            e_tiles[pj] = epool.tile([128, 2, TB], F8,
                                                     name="e8", tag="e8")
                        e8t = e_tiles[pj]
                        if i == NTB - 1:
                            stag = ["s_ps", "qkv0", "qkv1", "qkv2"][kt % 4]
                        else:
                            stag = "s_ps"
                        if stag == "s_ps":
                            s_ps = sps_pool.tile([128, TB], F32, name="s_ps")
                        else:
                            s_ps = qkvps.tile([128, TB], F32, name=stag,
                                              tag=stag)
                        j = kt - 4 * i
                        lo = 128 * j if j > 0 else 0
                        nc.tensor.matmul(s_ps[:, lo:TB], k8[:, :, ts(kt, 128)],
                                         q8[qh][:, :, i * TB + lo:
                                                (i + 1) * TB],
                                         start=True, stop=True, perf_mode=DR)
                        nc.scalar.activation(e8t[:, si, lo:TB],
                                             s_ps[:, lo:TB], AF.Exp,
                                             scale=SCALE)
                        if j >= 0:
                            if lo > 0:
                                # the skipped prefix holds stale pool bytes
                                # (can be fp8 NaN/Inf -- x*0 would keep NaN):
                                # zero it explicitly on the idle Pool engine
                                nc.gpsimd.memset(e8t[:, si, 0:lo], 0)
                            nc.vector.tensor_tensor(
                                out=e8t[:, si, lo:lo + 128],
                                in0=e8t[:, si, lo:lo + 128],
                                in1=mask_sb[:, j, lo:lo + 128], op=OP.mult)

                    def emit_consume_piece(ck, npair=npair, ctx_ps=ctx_ps,
                                           den_ps=den_ps, e_tiles=e_tiles):
                        pj = ck // 2
                        e8t = e_tiles[pj]
                        if ck % 2 == 0:
                            nc.tensor.matmul(ctx_ps[:], vhi[:, ts(pj, 2), :],
                                             e8t[:], start=(pj == 0),
                                             stop=False, perf_mode=DR)
                        else:
                            nc.tensor.matmul(ctx_ps[:], vlo[:, ts(pj, 2), :],
                                             e8t[:], start=False,
                                             stop=(pj == npair - 1),
                                             perf_mode=DR)
                            nc.tensor.matmul(den_ps[:], ones_sb[:], e8t[:],
                                             start=(pj == 0),
                                             stop=(pj == npair - 1),
                                             perf_mode=DR)

                    LAG = 4 if npair > 2 else 3
                    for k in range(nkt + LAG):
                        if k < nkt:
                            emit_score_kt(k)
                        ck = k - LAG
                        if ck >= 0:
                            emit_consume_piece(ck)
                            if ck % 2 == 1:
                                filler()

                    recip = t2pool.tile([128, TB], F32, tag="recip",
                                        name="recip")
                    nc.vector.reciprocal(recip[:], den_ps[:])
                    nc.vector.tensor_tensor(out=ctxT[qh][:, ts(i, TB)],
                                            in0=ctx_ps[:], in1=recip[:],
                                            op=OP.mult)

            # ---------------- main pipeline ----------------
            emit_h_dma(0, 0)
            nc.sync.dma_start(w_sb[:, 0:1, :], w_d[:, 0:1, :])
            emit_h_dma(0, 1)
            nc.sync.dma_start(w_sb[:, 1:4, :], w_d[:, 1:4, :])
            nc.sync.dma_start(cs_sb[:, :, ts(0, TB)], cs_d[:, :, ts(0, TB)])
            emit_h_dma(0, 2)
            nc.sync.dma_start(w_sb[:, 4:8, :], w_d[:, 4:8, :])
            emit_h_dma(0, 3)
            nc.sync.dma_start(w_sb[:, 8:12, :], w_d[:, 8:12, :])
            emit_h_dma(0, 4)
            nc.sync.dma_start(w_sb[:, 12:16, :], w_d[:, 12:16, :])
            for gi in range(len(GROUPS0)):
                emit_qkv_group(0, gi)
            emit_rope(0)
            for i in range(NTB):
                emit_attn_block(i)
                pending.extend((tt, n) for tt in range(4 * i, 4 * i + 4)
                               for n in range(4))
            while pending:
                emit_outproj_unit(tail=True)

    nc.compile()
    return nc


_NC_CACHE = None


def _get_nc():
    global _NC_CACHE
    if _NC_CACHE is None:
        _NC_CACHE = _build()
    return _NC_CACHE


def _host_tables(position_ids: np.ndarray):
    pos = np.asarray(position_ids, np.float32)
    inv_freq = (1.0 / (THETA ** (np.arange(0, D, 2, dtype=np.float32) / D)))
    ang = pos[:, None] * inv_freq[None, :]          # [T, 64] f32
    cos = np.cos(ang).T                             # [64, T]
    sin = np.sin(ang).T
    cosT = np.concatenate([cos, cos], axis=0).astype(np.float16)
    sinT = np.concatenate([-sin, sin], axis=0).astype(np.float16)
    return cosT, sinT


def _host_masks():
    # mask for diagonal tile j (keys 128j..128j+128 of the block): columns
    # [0, 128(j+1)): zero where q < k, i.e. col < 128j + row
    r = np.arange(128)[:, None]
    c = np.arange(TB)[None, :]
    m = np.stack([(c - r - 128 * j >= 0) for j in range(4)], axis=1)
    return m.astype(np.float16)                     # [128, 4, TB]


def kernel(hidden_states, position_ids, Wqkv, Wo):
    hidden_states = np.asarray(hidden_states, np.float32)
    Wqkv = np.asarray(Wqkv, np.float32)
    Wo = np.asarray(Wo, np.float32)

    nc = _get_nc()

    hT16 = np.ascontiguousarray(hidden_states.T).astype(np.float16)
    cosT, sinT = _host_tables(position_ids)
    cs16 = np.ascontiguousarray(np.stack([cosT, sinT], axis=1))  # [128,2,T]
    masks = _host_masks()
    ones8 = np.ones((128, 2, 128), dtype=F8NP)

    wq = Wqkv[:, : H * D]
    wk = Wqkv[:, H * D: (H + KV) * D]
    wv = Wqkv[:, (H + KV) * D:]

    in_maps = []
    for c in range(N_CORES):
        kvh = (c * QH) // (H // KV)
        w_cols = np.concatenate(
            [wq[:, (c * QH) * D: (c * QH + 1) * D],
             wq[:, (c * QH + 1) * D: (c * QH + 2) * D],
             wk[:, kvh * D: (kvh + 1) * D],
             wv[:, kvh * D: (kvh + 1) * D]], axis=1)         # [HID, 512]
        w16 = np.ascontiguousarray(
            w_cols.reshape(HCN, 128, 4 * 128).transpose(1, 0, 2)
        ).astype(np.float16)                                 # [128, HCN, 512]
        wo_local = Wo[c * QH * D: (c + 1) * QH * D, :]       # [256, HID]
        wo16 = np.ascontiguousarray(
            wo_local.reshape(2, 128, HID).transpose(1, 0, 2)
        ).astype(np.float16)                                 # [128, 2, HID]
        in_maps.append({
            "hT16": hT16, "w16": w16, "cs16": cs16,
            "mask16": masks, "ones8": ones8, "wo16": wo16,
        })

    res = bass_utils.run_bass_kernel_spmd(nc, in_maps,
                                          core_ids=list(range(N_CORES)))
    parts = np.stack([res.results[c]["out16"].astype(np.float32)
                      for c in range(N_CORES)], 0)
    return parts.sum(axis=0, dtype=np.float32)


# revision 47
# speedup vs baseline: 1.4146x; 1.0384x over previous
"""Bass/Trainium2 kernel for BailingAttention (GQA prefill, causal, RoPE).

Sharding: tensor-parallel over heads across 8 NeuronCores. Each core computes
2 query heads + its group's shared KV head end-to-end (QKV projection, RoPE,
causal attention, output projection) and writes a partial [T, HID] fp16
output; the host sums the 8 partials (the row-parallel all-reduce).

Precision plan (gate is rel-err < 2e-2; this lands ~1.4e-2):
  - QKV + output projections: fp16 x fp16 matmuls (1 PE cycle/row).
  - Scores: fp8 DoubleRow, one instruction per key tile computing
    k8^T(q_hi + q_lo) -- the two DoubleRow subtile slots carry a hi/lo fp8
    split of q, so the q side is ~exact and only k carries fp8 noise.
    0.5 cycles/row: 2x fp32r.
  - exp: ACT engine writes fp8e4 directly; softmax numerator/denominator use
    the SAME quantized e so weight-quantization largely cancels.
  - PV: two DoubleRow instructions per key-tile PAIR: (v_hi[2j],v_hi[2j+1])
    and (v_lo[...]) against the e pair -- v is hi/lo-split (~exact), e noise
    cancels through the denominator. 2x fp32r.
  - Denominator: DoubleRow over e pairs with a ones stationary: 4x fp32r.

Schedule: one fused software pipeline. Block 0's QKV runs first; thereafter
attention for block i runs with the QKV matmuls of block i+1 and the
output-projection units of block i-1 woven between its score/PV steps as
tensor-engine filler, so the PE stays dense (and at full p-state clock)
while ACT works through the exps. All of block i+1's hidden-stream DMAs are
prefetched at the start of attention i; RoPE for block i+1 is emitted as
soon as its last hc lands so its latency hides under block i's remaining
pairs. Output-projection units rotate over four PSUM banks (the scratch
bank plus the three QKV banks, which are idle between accumulations).

Layouts on device (partition dim first):
  hT16    [HID, T] fp16 (host-transposed)  -> moving operand of QKV matmuls
  q8      [D, 2(hi,lo), T] fp8 per head    -> scores moving
  k8      [D, 2(dup), T] fp8               -> scores stationary slices
  v hi/lo [T-part, kt, D] fp8 natural      -> PV stationary; produced by
          per-token-tile matmuls (stationary = hT slice) -- no PE transpose
  e8      [kt, 2, TB] fp8 pair tiles       -> PV/denominator moving
  ctxT    [D, T] fp16                      -> output projection stationary
"""

import numpy as np
import ml_dtypes

import concourse.bass as bass
import concourse.mybir as mybir
import concourse.tile as tile
from concourse import bacc, bass_utils
from concourse.bass import ts

F32 = mybir.dt.float32
F16 = mybir.dt.float16
F8 = mybir.dt.float8e4
AF = mybir.ActivationFunctionType
OP = mybir.AluOpType
DR = mybir.MatmulPerfMode.DoubleRow

H, KV, D, HID, T = 16, 4, 128, 2048, 2048
THETA = 10000.0
N_CORES = 8
QH = H // N_CORES            # query heads per core = 2
TB = 512                     # token block
NTB = T // TB                # 4
HCN = HID // 128             # 16 h-chunks
SCALE = float(D) ** -0.5
F8NP = ml_dtypes.float8_e4m3

# hc group layout: block 0 ramps up (small first DMA so the first matmul
# starts early); other blocks use 4-chunk groups
GROUPS0 = [(0, 2), (2, 2), (4, 4), (8, 4), (12, 4)]
GROUPS = [(0, 4), (4, 4), (8, 4), (12, 4)]


def _build():
    nc = bacc.Bacc("TRN2", target_bir_lowering=False, debug=False,
                   num_devices=N_CORES)

    hT_d = nc.dram_tensor("hT16", [HID, T], F16, kind="ExternalInput").ap()
    w_d = nc.dram_tensor("w16", [128, HCN, 4 * 128], F16, kind="ExternalInput").ap()
    cs_d = nc.dram_tensor("cs16", [128, 2, T], F16, kind="ExternalInput").ap()
    mask_d = nc.dram_tensor("mask16", [128, 4, TB], F16, kind="ExternalInput").ap()
    ones_d = nc.dram_tensor("ones8", [128, 2, 128], F8, kind="ExternalInput").ap()
    wo_d = nc.dram_tensor("wo16", [128, 2, HID], F16, kind="ExternalInput").ap()
    out_d = nc.dram_tensor("out16", [T, HID], F16, kind="ExternalOutput").ap()

    hT_view = hT_d.rearrange("(hc p) t -> hc p t", p=128)

    with tile.TileContext(nc) as tc:
        with tc.tile_pool(name="const", bufs=1) as cpool, \
             tc.tile_pool(name="acts", bufs=1) as apool, \
             tc.tile_pool(name="hstream", bufs=8) as hpool, \
             tc.tile_pool(name="rtmp", bufs=3) as tpool, \
             tc.tile_pool(name="p2e", bufs=10) as epool, \
             tc.tile_pool(name="p2tmp", bufs=2) as t2pool, \
             tc.tile_pool(name="p3out", bufs=3) as opool, \
             tc.tile_pool(name="qkvps", bufs=1, space="PSUM") as qkvps, \
             tc.tile_pool(name="scrps", bufs=1, space="PSUM") as scrps, \
             tc.tile_pool(name="sps", bufs=1, space="PSUM") as sps_pool, \
             tc.tile_pool(name="ops", bufs=1, space="PSUM") as ops_pool, \
             tc.tile_pool(name="cps", bufs=1, space="PSUM") as cps_pool, \
             tc.tile_pool(name="dps", bufs=1, space="PSUM") as dps_pool:

            w_sb = cpool.tile([128, HCN, 4 * 128], F16)
            cs_sb = cpool.tile([128, 2, T], F16)
            mask_sb = cpool.tile([128, 4, TB], F16)
            ones_sb = cpool.tile([128, 2, 128], F8)
            wo_sb = cpool.tile([128, 2, HID], F16)

            q8 = [apool.tile([128, 2, T], F8, name=f"q8_{i}", tag=f"q8_{i}")
                  for i in range(QH)]
            k8 = apool.tile([128, 2, T], F8)
            vhi = apool.tile([128, 16, 128], F8)
            vlo = apool.tile([128, 16, 128], F8)
            ctxT = [apool.tile([128, T], F16, name=f"ctxT{i}", tag=f"ctxT{i}")
                    for i in range(QH)]

            # ---------------- emit helpers ----------------
            qkv_live = {}   # block -> [ps_q0, ps_q1, ps_k]
            vnat_live = {}  # block -> ps_v (from scratch pool)
            h_tiles = {}    # (block, group-idx) -> sbuf tile
            h_keep = {}     # retained h tiles for deferred v-nat

            def emit_h_dma(i, gi):
                hc0, wid = (GROUPS0 if i == 0 else GROUPS)[gi]
                hT_t = hpool.tile([128, wid, TB], F16, tag=f"h{wid}")
                src = hT_view[hc0:hc0 + wid, :, ts(i, TB)]
                nc.sync.dma_start(hT_t[:], src.rearrange("g p t -> p g t"))
                h_tiles[(i, gi)] = hT_t

            def emit_qkv_group(i, gi):
                hc0, wid = (GROUPS0 if i == 0 else GROUPS)[gi]
                if gi == 0:
                    qkv_live[i] = [qkvps.tile([128, TB], F32, name=f"psqkv{n}",
                                              tag=f"qkv{n}") for n in range(3)]
                ps_qkv = qkv_live[i]
                hT_t = h_tiles.pop((i, gi))
                h_keep[(i, gi)] = hT_t
                for j in range(wid):
                    hc = hc0 + j
                    for n in range(3):
                        nc.tensor.matmul(ps_qkv[n][:], w_sb[:, hc, ts(n, 128)],
                                         hT_t[:, j, :], start=(hc == 0),
                                         stop=(hc == HCN - 1))

            def emit_vnat(i, tts=(0, 1, 2, 3)):
                # v in natural layout from the retained h tiles. tt-OUTER:
                # each PSUM sub-region's accumulation group must run
                # start->stop consecutively (interleaved same-bank
                # accumulation groups produce garbage on hardware).
                if i not in vnat_live:
                    vnat_live[i] = scrps.tile([128, 4, 128], F32, name="scr",
                                              tag="scr")
                ps_v = vnat_live[i]
                groups = GROUPS0 if i == 0 else GROUPS
                for tt in tts:
                    for gi, (hc0, wid) in enumerate(groups):
                        hT_t = h_keep[(i, gi)]
                        for j in range(wid):
                            hc = hc0 + j
                            nc.tensor.matmul(ps_v[:, tt, :],
                                             hT_t[:, j, ts(tt, 128)],
                                             w_sb[:, hc, ts(3, 128)],
                                             start=(hc == 0),
                                             stop=(hc == HCN - 1))
                if tts[-1] == 3:
                    for gi in range(len(groups)):
                        h_keep.pop((i, gi))
                    # v hi/lo casts
                    nc.vector.tensor_copy(vhi[:, ts(i, 4), :], ps_v[:])
                    nc.vector.tensor_tensor(out=vlo[:, ts(i, 4), :],
                                            in0=ps_v[:],
                                            in1=vhi[:, ts(i, 4), :],
                                            op=OP.subtract)
                    vnat_live.pop(i)

            def emit_rope(i):
                ps_qkv = qkv_live.pop(i)
                # fp16 RoPE; k chunk (slot 2) first so scores can start
                # after k + q0 land
                x_all = tpool.tile([128, 3, TB], F16, tag="ropex")
                for n in (2, 0, 1):
                    nc.vector.tensor_copy(x_all[:, n, :], ps_qkv[n][:])
                xsw = tpool.tile([128, 3, TB], F16, tag="ropesw")
                nc.scalar.dma_start(xsw[0:64, :, :], x_all[64:128, :, :])
                nc.scalar.dma_start(xsw[64:128, :, :], x_all[0:64, :, :])
                for n in (2, 0, 1):
                    t2 = tpool.tile([128, TB], F16, tag="ropet2")
                    nc.vector.tensor_tensor(out=t2[:], in0=xsw[:, n, :],
                                            in1=cs_sb[:, 1, ts(i, TB)],
                                            op=OP.mult)
                    m1 = tpool.tile([128, TB], F16, tag="ropem1")
                    nc.gpsimd.tensor_tensor(out=m1[:], in0=x_all[:, n, :],
                                            in1=cs_sb[:, 0, ts(i, TB)],
                                            op=OP.mult)
                    if n < QH:
                        qf = tpool.tile([128, TB], F16, tag="ropeqf")
                        nc.vector.tensor_tensor(out=qf[:], in0=m1[:], in1=t2[:],
                                                op=OP.add)
                        nc.gpsimd.tensor_copy(q8[n][:, 0, ts(i, TB)], qf[:])
                        nc.vector.tensor_tensor(
                            out=q8[n][:, 1, ts(i, TB)], in0=qf[:],
                            in1=q8[n][:, 0, ts(i, TB)], op=OP.subtract)
                    else:
                        nc.vector.tensor_tensor(out=k8[:, 0, ts(i, TB)],
                                                in0=m1[:], in1=t2[:], op=OP.add)
                        nc.scalar.copy(k8[:, 1, ts(i, TB)], k8[:, 0, ts(i, TB)])

            vnat_q = []     # deferred v-nat half-emissions
            pending = []    # outproj units (tt, n) awaiting emission
            osb_map = {}
            unit_ctr = [0]
            TAIL_TAGS = ["ops", "qkv0", "qkv1", "qkv2"]

            def emit_outproj_unit(tail=False, dve_only=False):
                tt, n = pending.pop(0)
                if tt not in osb_map:
                    osb_map[tt] = opool.tile([128, HID], F16, name="o_sb")
                o_sb = osb_map[tt]
                unit_ctr[0] += 1
                if tail:
                    tag = TAIL_TAGS[unit_ctr[0] % 4]
                else:
                    tag = ["ops", "scr"][unit_ctr[0] % 2]
                if tag == "ops":
                    ps_o = ops_pool.tile([128, TB], F32, name="ps_o", tag="ops")
                elif tag == "scr":
                    ps_o = scrps.tile([128, 4, 128], F32, name="scr", tag="scr")
                else:
                    ps_o = qkvps.tile([128, TB], F32, name=f"ps{tag}", tag=tag)
                nc.tensor.matmul(ps_o[:], ctxT[0][:, ts(tt, 128)],
                                 wo_sb[:, 0, ts(n, 512)], start=True, stop=False)
                nc.tensor.matmul(ps_o[:], ctxT[1][:, ts(tt, 128)],
                                 wo_sb[:, 1, ts(n, 512)], start=False, stop=True)
                if dve_only or unit_ctr[0] % 2 == 0:
                    nc.vector.tensor_copy(o_sb[:, ts(n, 512)], ps_o[:])
                else:
                    nc.scalar.copy(o_sb[:, ts(n, 512)], ps_o[:])
                if n == 3:
                    nc.sync.dma_start(out_d[ts(tt, 128), :], o_sb[:])
                    del osb_map[tt]

            def emit_attn_block(i):
                """Attention for block i, weaving in QKV matmuls of block
                i+1 and outproj units of block i-1 as PE filler."""
                ngrp = len(GROUPS) if i < NTB - 1 else 0
                grp_q = list(range(ngrp))
                if i < NTB - 1:
                    for gi in range(ngrp):
                        emit_h_dma(i + 1, gi)
                    nc.sync.dma_start(cs_sb[:, :, ts(i + 1, TB)],
                                      cs_d[:, :, ts(i + 1, TB)])
                if i == 0:
                    nc.sync.dma_start(mask_sb[:], mask_d)
                    nc.sync.dma_start(ones_sb[:], ones_d)
                    nc.sync.dma_start(wo_sb[:], wo_d)
                if i == 0:
                    # block 0's v-nat runs here, out of its DMA-bound QKV
                    # stream, covering RoPE(0)'s latency tail
                    emit_vnat(0)
                npair = 2 * (i + 1)
                steps_total = npair * QH
                step = [0]

                def filler():
                    steps_left = steps_total - step[0]
                    step[0] += 1
                    if grp_q:
                        for _ in range(2 if i <= 1 else 1):
                            if grp_q:
                                emit_qkv_group(i + 1, grp_q.pop(0))
                        if not grp_q:
                            emit_rope(i + 1)
                            vnat_q.extend([(i + 1, (0, 1)), (i + 1, (2, 3))])
                        return
                    if vnat_q:
                        blk, tts = vnat_q.pop(0)
                        emit_vnat(blk, tts)
                        return
                    if pending and steps_left > 0:
                        n_fill = (len(pending) + steps_left - 1) // steps_left
                        for _ in range(min(n_fill, len(pending))):
                            emit_outproj_unit(tail=(i == NTB - 1),
                                              dve_only=(i == NTB - 1))

                for qh in range(QH):
                    ctx_ps = cps_pool.tile([128, TB], F32, name="ctx_ps")
                    den_ps = dps_pool.tile([128, TB], F32, name="den_ps")
                    nkt = 4 * (i + 1)
                    e_tiles = [None] * npair

                    def emit_score_kt(kt, i=i, qh=qh, e_tiles=e_tiles):
                        pj, si = kt // 2, kt % 2
                        if si == 0:
                            e_tiles[pj] = epool.tile([128, 2, TB], F8,
                                                     name="e8", tag="e8")
                        e8t = e_tiles[pj]
                        if i == NTB - 1:
                            stag = ["s_ps", "qkv0", "qkv1", "qkv2"][kt % 4]
                        else:
                            stag = "s_ps"
                        if stag == "s_ps":
                            s_ps = sps_pool.tile([128, TB], F32, name="s_ps")
                        else:
                            s_ps = qkvps.tile([128, TB], F32, name=stag,
                                              tag=stag)
                        j = kt - 4 * i
                        lo = 128 * j if j > 0 else 0
                        nc.tensor.matmul(s_ps[:, lo:TB], k8[:, :, ts(kt, 128)],
                                         q8[qh][:, :, i * TB + lo:
                                                (i + 1) * TB],
                                         start=True, stop=True, perf_mode=DR)
                        nc.scalar.activation(e8t[:, si, lo:TB],
                                             s_ps[:, lo:TB], AF.Exp,
                                             scale=SCALE)
                        if j >= 0:
                            if lo > 0:
                                # the skipped prefix holds stale pool bytes
                                # (can be fp8 NaN/Inf -- x*0 would keep NaN):
                                # zero it explicitly on the idle Pool engine
                                nc.gpsimd.memset(e8t[:, si, 0:lo], 0)
                            nc.vector.tensor_tensor(
                                out=e8t[:, si, lo:lo + 128],
                                in0=e8t[:, si, lo:lo + 128],
                                in1=mask_sb[:, j, lo:lo + 128], op=OP.mult)

                    def emit_consume_piece(ck, npair=npair, ctx_ps=ctx_ps,
                                           den_ps=den_ps, e_tiles=e_tiles):
                        pj = ck // 2
                        e8t = e_tiles[pj]
                        if ck % 2 == 0:
                            nc.tensor.matmul(ctx_ps[:], vhi[:, ts(pj, 2), :],
                                             e8t[:], start=(pj == 0),
                                             stop=False, perf_mode=DR)
                        else:
                            nc.tensor.matmul(ctx_ps[:], vlo[:, ts(pj, 2), :],
                                             e8t[:], start=False,
                                             stop=(pj == npair - 1),
                                             perf_mode=DR)
                            nc.tensor.matmul(den_ps[:], ones_sb[:], e8t[:],
                                             start=(pj == 0),
                                             stop=(pj == npair - 1),
                                             perf_mode=DR)

                    LAG = 4 if npair > 2 else 3
                    for k in range(nkt + LAG):
                        if k < nkt:
                            emit_score_kt(k)
                        ck = k - LAG
                        if ck >= 0:
                            emit_consume_piece(ck)
                            if ck % 2 == 1:
                                filler()

                    recip = t2pool.tile([128, TB], F32, tag="recip",
                                        name="recip")
                    nc.vector.reciprocal(recip[:], den_ps[:])
                    nc.vector.tensor_tensor(out=ctxT[qh][:, ts(i, TB)],
                                            in0=ctx_ps[:], in1=recip[:],
                                            op=OP.mult)

            # ---------------- main pipeline ----------------
            emit_h_dma(0, 0)
            nc.sync.dma_start(w_sb[:, 0:2, :], w_d[:, 0:2, :])
            emit_h_dma(0, 1)
            nc.sync.dma_start(w_sb[:, 2:4, :], w_d[:, 2:4, :])
            nc.sync.dma_start(cs_sb[:, :, ts(0, TB)], cs_d[:, :, ts(0, TB)])
            emit_h_dma(0, 2)
            nc.sync.dma_start(w_sb[:, 4:8, :], w_d[:, 4:8, :])
            emit_h_dma(0, 3)
            nc.sync.dma_start(w_sb[:, 8:12, :], w_d[:, 8:12, :])
            emit_h_dma(0, 4)
            nc.sync.dma_start(w_sb[:, 12:16, :], w_d[:, 12:16, :])
            for gi in range(len(GROUPS0)):
                emit_qkv_group(0, gi)
            emit_rope(0)
            for i in range(NTB):
                emit_attn_block(i)
                pending.extend((tt, n) for tt in range(4 * i, 4 * i + 4)
                               for n in range(4))
            while pending:
                emit_outproj_unit(tail=True)

    nc.compile()
    return nc


_NC_CACHE = None


def _get_nc():
    global _NC_CACHE
    if _NC_CACHE is None:
        _NC_CACHE = _build()
    return _NC_CACHE


def _host_tables(position_ids: np.ndarray):
    pos = np.asarray(position_ids, np.float32)
    inv_freq = (1.0 / (THETA ** (np.arange(0, D, 2, dtype=np.float32) / D)))
    ang = pos[:, None] * inv_freq[None, :]          # [T, 64] f32
    cos = np.cos(ang).T                             # [64, T]
    sin = np.sin(ang).T
    cosT = np.concatenate([cos, cos], axis=0).astype(np.float16)
    sinT = np.concatenate([-sin, sin], axis=0).astype(np.float16)
    return cosT, sinT


def _host_masks():
    # mask for diagonal tile j (keys 128j..128j+128 of the block): columns
    # [0, 128(j+1)): zero where q < k, i.e. col < 128j + row
    r = np.arange(128)[:, None]
    c = np.arange(TB)[None, :]
    m = np.stack([(c - r - 128 * j >= 0) for j in range(4)], axis=1)
    return m.astype(np.float16)                     # [128, 4, TB]


def kernel(hidden_states, position_ids, Wqkv, Wo):
    hidden_states = np.asarray(hidden_states, np.float32)
    Wqkv = np.asarray(Wqkv, np.float32)
    Wo = np.asarray(Wo, np.float32)

    nc = _get_nc()

    hT16 = np.ascontiguousarray(hidden_states.T).astype(np.float16)
    cosT, sinT = _host_tables(position_ids)
    cs16 = np.ascontiguousarray(np.stack([cosT, sinT], axis=1))  # [128,2,T]
    masks = _host_masks()
    ones8 = np.ones((128, 2, 128), dtype=F8NP)

    wq = Wqkv[:, : H * D]
    wk = Wqkv[:, H * D: (H + KV) * D]
    wv = Wqkv[:, (H + KV) * D:]

    in_maps = []
    for c in range(N_CORES):
        kvh = (c * QH) // (H // KV)
        w_cols = np.concatenate(
            [wq[:, (c * QH) * D: (c * QH + 1) * D],
             wq[:, (c * QH + 1) * D: (c * QH + 2) * D],
             wk[:, kvh * D: (kvh + 1) * D],
             wv[:, kvh * D: (kvh + 1) * D]], axis=1)         # [HID, 512]
        w16 = np.ascontiguousarray(
            w_cols.reshape(HCN, 128, 4 * 128).transpose(1, 0, 2)
        ).astype(np.float16)                                 # [128, HCN, 512]
        wo_local = Wo[c * QH * D: (c + 1) * QH * D, :]       # [256, HID]
        wo16 = np.ascontiguousarray(
            wo_local.reshape(2, 128, HID).transpose(1, 0, 2)
        ).astype(np.float16)                                 # [128, 2, HID]
        in_maps.append({
            "hT16": hT16, "w16": w16, "cs16": cs16,
            "mask16": masks, "ones8": ones8, "wo16": wo16,
        })

    res = bass_utils.run_bass_kernel_spmd(nc, in_maps,
                                          core_ids=list(range(N_CORES)))
    parts = np.stack([res.results[c]["out16"].astype(np.float32)
                      for c in range(N_CORES)], 0)
    return parts.sum(axis=0, dtype=np.float32)
